# revision 1
# baseline (speedup 1.0000x reference)
"""GCNConv-variant Trainium2 kernel (8 NeuronCores, SPMD via bass/tile).

Math (from the reference):
    deg  = in-degree of col over all edges               [N]
    dis  = where(deg>0, deg^-1/2, 0)                     [N]
    pp   = sigmoid(p) + 1
    mu   = min(x)
    x1   = (x - mu + 1e-6)^pp                            [N,128]
    agg[i] = sum_{e: row[e]==i} dis[row]*dis[col]*x1[col[e]]
    out  = (agg + 1e-6)^(1/pp) + (1+eps)*x + mu

Distribution (row-sharded destination ranges, 3 launches):
    P0 (uniform SPMD program, 8 cores): per-core x-slice min; in-degree of
       the core's owned node range via dma_scatter_add histogram; dis.
    host: mu = min of the 8 partial minima (pure gather/reduce of shards).
    P1 (uniform SPMD program): y = dis * (x - mu + 1e-6)^pp for the owned
       node range, emitted as a bf16 hi/lo pair [rows, 256].
    host: concatenate the 8 y-slices, replicate to all cores (the
       sharding_hint's "replicate x" pattern, applied to y).
    P2 (one program per core, concurrent on the 8 devices): gather y rows
       by col via SWDGE dma_gather; one-hot bf16 matmuls (P^T @ [y_hi|y_lo])
       accumulate per-row-window segment sums in PSUM; fused output
       transform (ln/exp powers + (1+eps)x + mu) written per window.
"""

import math
import os
import sys
from contextlib import ExitStack

sys.path.insert(0, "/opt/trn_rl_repo")

import numpy as np
import ml_dtypes

import concourse.bass as bass
import concourse.bacc as bacc
import concourse.bass_isa as bass_isa
import concourse.mybir as mybir
import concourse.tile as tile

F32 = mybir.dt.float32
BF16 = mybir.dt.bfloat16
I16 = mybir.dt.int16
I32 = mybir.dt.int32
EPS_NUM = 1e-6
ALU = mybir.AluOpType
ACT = mybir.ActivationFunctionType


class Cfg:
    def __init__(self, N=100000, E=1600000, D=128, ncores=8, bank_rows=25000,
                 chunk=8, group=4):
        assert D == 128
        self.N, self.E, self.D, self.ncores = N, E, D, ncores
        self.rpc_real = N // ncores            # owned rows per core
        assert self.rpc_real * ncores == N
        self.rpc = ((self.rpc_real + 127) // 128) * 128   # padded rows
        self.nwin = self.rpc // 128            # row windows per core
        self.bank_rows = bank_rows             # gather bank size (int16 limit)
        assert bank_rows <= 32768
        self.nbanks = (N + bank_rows - 1) // bank_rows
        self.chunk = chunk                     # gather batches per SWDGE call
        self.group = group                     # windows per PSUM bank tile


# ----------------------------------------------------------------------------
# host-side planning (pure data movement / layout; no reference math)
# ----------------------------------------------------------------------------

def _wrap_idxs(idx_linear):
    """SWDGE index layout: slot i lives at [i%16, i//16], tiled to 128 parts."""
    n = len(idx_linear)
    assert n % 16 == 0
    a = np.zeros((16, n // 16), np.int16)
    ar = np.arange(n)
    a[ar % 16, ar // 16] = idx_linear.astype(np.int16)
    return np.tile(a, (8, 1))


class Batch:
    __slots__ = ("bank", "win", "sec", "rl", "chunk_id", "chunk_col")

    def __init__(self, bank, win, sec, rl):
        self.bank, self.win, self.sec, self.rl = bank, win, sec, rl


class CorePlan:
    pass


def plan_core_p2(rows_local, cols, cfg: Cfg):
    """Plan one core's P2 schedule. rows_local in [0, rpc_real)."""
    bank = cols // cfg.bank_rows
    order = np.lexsort((rows_local, bank))
    r = rows_local[order]
    c = (cols - bank * cfg.bank_rows)[order]
    bk = bank[order]

    batches = []
    idx_parts = []
    for b in range(cfg.nbanks):
        lo = np.searchsorted(bk, b, "left")
        hi = np.searchsorted(bk, b, "right")
        rb, cb = r[lo:hi], c[lo:hi]
        i = 0
        while i < len(rb):
            w = int(rb[i]) // 128
            j = min(i + 128, len(rb))
            # keep rows within windows [w, w+1] only
            cut = int(np.searchsorted(rb[i:j], (w + 2) * 128, "left"))
            j = i + cut
            take = j - i
            idx = np.concatenate([cb[i:j], np.zeros(128 - take, np.int64)])
            rl = np.concatenate([rb[i:j] - w * 128,
                                 np.full(128 - take, -1, np.int64)])
            sec = bool((rb[i:j] >= (w + 1) * 128).any())
            batches.append(Batch(b, w, sec, rl.astype(np.float32)))
            idx_parts.append(idx)
            i = j

    nb = len(batches)
    plan = CorePlan()
    plan.nbatches = nb
    if nb == 0:
        plan.idx_wrapped = np.zeros((128, 8), np.int16)
        plan.row_local = np.zeros((128, 1), np.float32)
        plan.batches = []
        plan.chunks = []
        plan.flushes = {}
        plan.touch = {}
        return plan

    idx_all = np.concatenate(idx_parts)            # nb*128 slots
    plan.idx_wrapped = _wrap_idxs(idx_all)
    rlm = np.stack([bt.rl for bt in batches], axis=1)   # [128, nb]
    plan.row_local = rlm.astype(np.float32)
    plan.batches = batches

    # gather chunks: runs of batches within one bank, up to cfg.chunk each
    chunks = []   # (bank, slot0, nbatch)
    i = 0
    while i < nb:
        b = batches[i].bank
        j = i
        while j < nb and batches[j].bank == b and j - i < cfg.chunk:
            j += 1
        chunks.append((b, i, j - i))
        for k in range(i, j):
            batches[k].chunk_id = len(chunks) - 1
            batches[k].chunk_col = k - i
        i = j
    plan.chunks = chunks

    # per-(bank,group) PSUM tile bookkeeping: matmul targets & flush points
    # touch[(bank, grp)] = ordered list of (batch_idx, which, win)
    touch = {}
    for bi, bt in enumerate(batches):
        tgt = [(bt.win, "lo")]
        if bt.sec:
            tgt.append((bt.win + 1, "hi"))
        for w, which in tgt:
            key = (bt.bank, w // cfg.group)
            touch.setdefault(key, []).append((bi, which, w))
    plan.touch = touch

    # flush after the last batch touching (bank, grp); record touched windows
    flushes = {}   # batch_idx -> list of (bank, grp, [touched windows sorted])
    for (b, g), lst in touch.items():
        last_bi = max(e[0] for e in lst)
        wins = sorted({e[2] for e in lst})
        flushes.setdefault(last_bi, []).append((b, g, wins))
    plan.flushes = flushes

    # start/stop flags: for each (bank, win): first and last (batch, which)
    first_touch = {}
    last_touch = {}
    for (b, g), lst in touch.items():
        for bi, which, w in lst:
            key = (b, w)
            if key not in first_touch:
                first_touch[key] = (bi, which)
            last_touch[key] = (bi, which)
    plan.first_touch = first_touch
    plan.last_touch = last_touch
    return plan


def plan_core_p0(cols_local, cfg: Cfg):
    """One-hot histogram batching over the core's owned-range cols (sorted)."""
    v = np.sort(cols_local)
    batches = []   # (win, sec)
    rls = []
    i = 0
    while i < len(v):
        w = int(v[i]) // 128
        j = min(i + 128, len(v))
        cut = int(np.searchsorted(v[i:j], (w + 2) * 128, "left"))
        j = i + cut
        take = j - i
        rl = np.concatenate([v[i:j] - w * 128, np.full(128 - take, -1, np.int64)])
        sec = bool((v[i:j] >= (w + 1) * 128).any())
        batches.append((w, sec))
        rls.append(rl.astype(np.float32))
        i = j

    plan = CorePlan()
    nb = len(batches)
    plan.nbatches = nb
    if nb == 0:
        plan.row_local = np.zeros((128, 1), np.float32)
        plan.batches = []
        plan.flushes = {}
        plan.first_touch = {}
        plan.last_touch = {}
        return plan
    plan.row_local = np.stack(rls, axis=1)
    plan.batches = batches

    touch = {}
    for bi, (w, sec) in enumerate(batches):
        tgt = [(w, "lo")] + ([(w + 1, "hi")] if sec else [])
        for tw, which in tgt:
            touch.setdefault(tw // cfg.group, []).append((bi, which, tw))
    flushes = {}
    first_touch, last_touch = {}, {}
    for g, lst in touch.items():
        last_bi = max(e[0] for e in lst)
        wins = sorted({e[2] for e in lst})
        flushes.setdefault(last_bi, []).append((g, wins))
        for bi, which, w in lst:
            if w not in first_touch:
                first_touch[w] = (bi, which)
            last_touch[w] = (bi, which)
    plan.flushes = flushes
    plan.first_touch = first_touch
    plan.last_touch = last_touch
    return plan


def plan_all(edge_index, cfg: Cfg):
    """Shard edges; returns per-core host data for P0 and P2."""
    row = np.asarray(edge_index[0])
    col = np.asarray(edge_index[1])

    # ---- P0: histogram shards (col buckets by owner range) ----
    owner = col // cfg.rpc_real
    p0_plans = []
    for cr in range(cfg.ncores):
        sel = owner == cr
        loc = (col[sel] - cr * cfg.rpc_real).astype(np.int64)
        p0_plans.append(plan_core_p0(loc, cfg))

    # ---- P2: row shards ----
    rowner = row // cfg.rpc_real
    plans = []
    for cr in range(cfg.ncores):
        sel = rowner == cr
        plans.append(plan_core_p2((row[sel] - cr * cfg.rpc_real).astype(np.int64),
                                  col[sel].astype(np.int64), cfg))
    return p0_plans, plans


# ----------------------------------------------------------------------------
# bass program builders
# ----------------------------------------------------------------------------

def _mk_nc():
    return bacc.Bacc("TRN2", target_bir_lowering=False, debug=False,
                     enable_partition_id=False)


def _iota_tiles():
    i = np.arange(128, dtype=np.float32)
    lo = np.tile(i, (128, 1)).astype(ml_dtypes.bfloat16)
    hi = np.tile(i + 128.0, (128, 1)).astype(ml_dtypes.bfloat16)
    return lo, hi


def build_p0(cfg: Cfg, plan: CorePlan, rep=0):
    """Per-core program: x-slice min + one-hot matmul degree histogram + dis."""
    dynamic = rep == -1
    nc = bacc.Bacc("TRN2", target_bir_lowering=False, debug=False,
                   enable_partition_id=False)
    NW, G = cfg.nwin, cfg.group
    nbat = max(plan.nbatches, 1)
    x = nc.dram_tensor("x_own", [cfg.rpc, 128], F32, kind="ExternalInput")
    rloc = nc.dram_tensor("col_local", [128, nbat], F32, kind="ExternalInput")
    iota_lo_d = nc.dram_tensor("iota_lo", [128, 128], BF16, kind="ExternalInput")
    iota_hi_d = nc.dram_tensor("iota_hi", [128, 128], BF16, kind="ExternalInput")
    if dynamic:
        rep_in = nc.dram_tensor("rep", [1, 1], I32, kind="ExternalInput")
    xmin = nc.dram_tensor("xmin", [1, 1], F32, kind="ExternalOutput")
    dis = nc.dram_tensor("dis_own", [cfg.rpc], F32, kind="ExternalOutput")

    with tile.TileContext(nc) as tc, ExitStack() as ctx:
        pool = ctx.enter_context(tc.tile_pool(name="p0", bufs=3))
        ppool = ctx.enter_context(tc.tile_pool(name="p0p", bufs=4))
        psum = ctx.enter_context(tc.tile_pool(name="p0s", bufs=4, space="PSUM"))
        cpool = ctx.enter_context(tc.tile_pool(name="p0c", bufs=1))

        rl_sb = cpool.tile([128, nbat], F32)
        nc.sync.dma_start(rl_sb[:], rloc.ap()[:])
        io_lo = cpool.tile([128, 128], BF16)
        nc.sync.dma_start(io_lo[:], iota_lo_d.ap()[:])
        io_hi = cpool.tile([128, 128], BF16)
        nc.sync.dma_start(io_hi[:], iota_hi_d.ap()[:])
        ones_sb = cpool.tile([128, 1], BF16)
        nc.vector.memset(ones_sb[:], 1.0)
        deg_flat = cpool.tile([1, NW * 128], F32)
        runmin = cpool.tile([128, 1], F32)

        loop_cm = ExitStack()
        if dynamic:
            rep_sb = cpool.tile([1, 1], I32)
            nc.sync.dma_start(rep_sb[:], rep_in.ap()[:])
            regs = []
            for e in mybir.ALL_ENGINES:
                regs.append(nc.alloc_register(e, f"repreg_{e.name}"))
            nc.regs_load(bass.RegisterHandles(tuple(regs)), rep_sb[0:1, 0:1])
            rep_val = bass.make_scalar_value(
                bass.RegisterHandles(tuple(regs)), min_val=0, max_val=1 << 20)
            loop_cm.enter_context(tc.For_i(0, rep_val, 1))
        ctx.enter_context(loop_cm)
        nc.vector.memset(deg_flat[:], 0.0)

        # ---- x min (4-window supertiles) ----
        SW = 4
        nt4 = (NW // SW) * SW * 128
        xt4 = x.ap()[0:nt4, :].rearrange("(t w p) f -> t p w f", p=128, w=SW)
        xt3 = x.ap().rearrange("(w p) f -> w p f", p=128)
        nc.vector.memset(runmin[:], 1e30)
        for t in range(NW // SW):
            xt = pool.tile([128, SW, 128], F32)
            nc.sync.dma_start(xt[:], xt4[t])
            red = pool.tile([128, 1], F32)
            nc.vector.tensor_reduce(red[:], xt[:], mybir.AxisListType.XY, ALU.min)
            nc.vector.tensor_tensor(runmin[:], runmin[:], red[:], ALU.min)
        for w in range((NW // SW) * SW, NW):
            xt1 = pool.tile([128, 128], F32, name="xt1", tag="xt1")
            nc.sync.dma_start(xt1[:], xt3[w])
            red1 = pool.tile([128, 1], F32, name="red1", tag="red1")
            nc.vector.tensor_reduce(red1[:], xt1[:], mybir.AxisListType.X, ALU.min)
            nc.vector.tensor_tensor(runmin[:], runmin[:], red1[:], ALU.min)
        negmin = cpool.tile([128, 1], F32)
        nc.vector.tensor_scalar(negmin[:], runmin[:], -1.0, None, ALU.mult)
        allmax = cpool.tile([128, 1], F32)
        nc.gpsimd.partition_all_reduce(allmax[:], negmin[:], 128,
                                       bass_isa.ReduceOp.max)
        minv = cpool.tile([1, 1], F32)
        nc.vector.tensor_scalar(minv[:], allmax[0:1, :], -1.0, None, ALU.mult)
        nc.sync.dma_start(xmin.ap()[:], minv[:])

        # ---- degree histogram: ones^T @ one-hot(col) accumulated per window ----
        live_psum = {}
        for bi in range(plan.nbatches):
            w0, sec = plan.batches[bi]
            tgt = [("lo", w0)] + ([("hi", w0 + 1)] if sec else [])
            for which, w in tgt:
                g = w // G
                if g not in live_psum:
                    live_psum[g] = psum.tile([1, G * 128], F32,
                                             name="dgrp", tag="dgrp")
                pt = live_psum[g]
                P = ppool.tile([128, 128], BF16, name="P0P", tag="P0P")
                nc.vector.tensor_scalar(P[:], io_lo[:] if which == "lo" else io_hi[:],
                                        rl_sb[:, bi:bi + 1], None, ALU.is_equal)
                first = plan.first_touch[w] == (bi, which)
                last = plan.last_touch[w] == (bi, which)
                sl = pt[:, (w % G) * 128:(w % G) * 128 + 128]
                nc.tensor.matmul(sl, ones_sb[:], P[:], start=first, stop=last)
            for (fg, wins) in plan.flushes.get(bi, []):
                pt = live_psum.pop(fg)
                runs = []
                for w in wins:
                    if runs and w == runs[-1][1]:
                        runs[-1][1] = w + 1
                    else:
                        runs.append([w, w + 1])
                for a, bnd in runs:
                    nc.vector.tensor_tensor(
                        deg_flat[:, a * 128:bnd * 128],
                        deg_flat[:, a * 128:bnd * 128],
                        pt[:, (a % G) * 128:(a % G) * 128 + (bnd - a) * 128],
                        ALU.add)

        # ---- dis = mask * sqrt(1/max(deg,1)) on the flat [1, rpc] layout ----
        mask = cpool.tile([1, NW * 128], F32)
        nc.vector.tensor_scalar(mask[:], deg_flat[:], 0.5, None, ALU.is_ge)
        nc.vector.tensor_scalar(deg_flat[:], deg_flat[:], 1.0, None, ALU.max)
        nc.vector.reciprocal(deg_flat[:], deg_flat[:])
        nc.scalar.activation(deg_flat[:], deg_flat[:], ACT.Sqrt)
        nc.vector.tensor_tensor(deg_flat[:], deg_flat[:], mask[:], ALU.mult)
        nc.sync.dma_start(dis.ap().rearrange("(o f) -> o f", o=1), deg_flat[:, :])
    nc.compile()
    return nc


def build_p1(cfg: Cfg, rep=0):
    """Uniform SPMD program: y hi/lo pair for the owned slice."""
    dynamic = rep == -1
    nc = _mk_nc()
    x = nc.dram_tensor("x_own", [cfg.rpc, 128], F32, kind="ExternalInput")
    dis = nc.dram_tensor("dis_own", [cfg.rpc], F32, kind="ExternalInput")
    mu = nc.dram_tensor("mu", [1, 1], F32, kind="ExternalInput")
    p_in = nc.dram_tensor("p", [1, 1], F32, kind="ExternalInput")
    if dynamic:
        rep_in = nc.dram_tensor("rep", [1, 1], I32, kind="ExternalInput")
    y = nc.dram_tensor("y_own", [cfg.rpc, 256], BF16, kind="ExternalOutput")

    NW = cfg.nwin
    with tile.TileContext(nc) as tc, ExitStack() as ctx:
        pool = ctx.enter_context(tc.tile_pool(name="p1", bufs=3))
        cpool = ctx.enter_context(tc.tile_pool(name="p1c", bufs=1))

        # scalars
        psb = cpool.tile([1, 1], F32)
        nc.sync.dma_start(psb[:], p_in.ap()[:])
        sig = cpool.tile([1, 1], F32)
        nc.scalar.activation(sig[:], psb[:], ACT.Sigmoid)
        ppb = cpool.tile([128, 1], F32)
        nc.gpsimd.partition_broadcast(ppb[:], sig[:])
        pp_vec = cpool.tile([128, 1], F32)
        nc.vector.tensor_scalar(pp_vec[:], ppb[:], 1.0, None, ALU.add)
        musb = cpool.tile([1, 1], F32)
        nc.sync.dma_start(musb[:], mu.ap()[:])
        mub = cpool.tile([128, 1], F32)
        nc.gpsimd.partition_broadcast(mub[:], musb[:])
        cvec = cpool.tile([128, 1], F32)   # 1e-6 - mu
        nc.vector.tensor_scalar(cvec[:], mub[:], -1.0, EPS_NUM, ALU.mult, ALU.add)

        dis_sb = cpool.tile([128, NW], F32)
        nc.sync.dma_start(dis_sb[:], dis.ap().rearrange("(w p) -> p w", p=128))
        disc = cpool.tile([128, NW], F32)
        nc.vector.tensor_scalar(disc[:], dis_sb[:], 1e-30, None, ALU.max)
        ldis = cpool.tile([128, NW], F32)
        nc.scalar.activation(ldis[:], disc[:], ACT.Ln)

        loop_cm = ExitStack()
        if dynamic:
            rep_sb = cpool.tile([1, 1], I32)
            nc.sync.dma_start(rep_sb[:], rep_in.ap()[:])
            regs = []
            for e in mybir.ALL_ENGINES:
                regs.append(nc.alloc_register(e, f"repreg_{e.name}"))
            nc.regs_load(bass.RegisterHandles(tuple(regs)), rep_sb[0:1, 0:1])
            rep_val = bass.make_scalar_value(
                bass.RegisterHandles(tuple(regs)), min_val=0, max_val=1 << 20)
            loop_cm.enter_context(tc.For_i(0, rep_val, 1))
        ctx.enter_context(loop_cm)
        # 4-window supertiles: one DMA in/out per 4 windows, fused DVE ops
        SW = 4
        nt4 = (NW // SW) * SW * 128
        xt4 = x.ap()[0:nt4, :].rearrange("(t w p) f -> t p w f", p=128, w=SW)
        yt4 = y.ap()[0:nt4, :].rearrange("(t w p) f -> t p w f", p=128, w=SW)
        for t in range(NW // SW):
            xt = pool.tile([128, SW, 128], F32)
            nc.sync.dma_start(xt[:], xt4[t])
            t1 = pool.tile([128, SW, 128], F32)
            nc.scalar.activation(t1[:], xt[:], ACT.Ln, bias=cvec[:, 0:1])
            yf = pool.tile([128, SW, 128], F32)
            for w in range(SW):
                nc.scalar.activation(yf[:, w, :], t1[:, w, :], ACT.Exp,
                                     bias=ldis[:, t * SW + w:t * SW + w + 1],
                                     scale=pp_vec[:, 0:1])
            yt = pool.tile([128, SW, 256], BF16)
            nc.vector.tensor_copy(yt[:, :, 0:128], yf[:])
            hi32 = pool.tile([128, SW, 128], F32)
            nc.vector.tensor_copy(hi32[:], yt[:, :, 0:128])
            nc.vector.tensor_tensor(yt[:, :, 128:256], yf[:], hi32[:],
                                    ALU.subtract)
            nc.sync.dma_start(yt4[t], yt[:])
        xt3 = x.ap().rearrange("(w p) f -> w p f", p=128)
        yt3 = y.ap().rearrange("(w p) f -> w p f", p=128)
        for w in range((NW // SW) * SW, NW):
            xt1 = pool.tile([128, 128], F32, name="xt1", tag="xt1")
            nc.sync.dma_start(xt1[:], xt3[w])
            t1b = pool.tile([128, 128], F32, name="t1b", tag="t1b")
            nc.scalar.activation(t1b[:], xt1[:], ACT.Ln, bias=cvec[:, 0:1])
            yfb = pool.tile([128, 128], F32, name="yfb", tag="yfb")
            nc.scalar.activation(yfb[:], t1b[:], ACT.Exp,
                                 bias=ldis[:, w:w + 1], scale=pp_vec[:, 0:1])
            ytb = pool.tile([128, 256], BF16, name="ytb", tag="ytb")
            nc.vector.tensor_copy(ytb[:, 0:128], yfb[:])
            hi32b = pool.tile([128, 128], F32, name="hi32b", tag="hi32b")
            nc.vector.tensor_copy(hi32b[:], ytb[:, 0:128])
            nc.vector.tensor_tensor(ytb[:, 128:256], yfb[:], hi32b[:],
                                    ALU.subtract)
            nc.sync.dma_start(yt3[w], ytb[:])
    nc.compile()
    return nc


def build_p2(cfg: Cfg, plan: CorePlan, rep=0):
    """Per-core program: gather + one-hot matmul segment-sum + output.

    rep=0: straight-line. rep>0 would be static replication (unused).
    dynamic=True via rep=-1: For_i loop with trip count from `rep` input.
    """
    dynamic = rep == -1
    nc = bacc.Bacc("TRN2", target_bir_lowering=False, debug=False,
                   enable_partition_id=False, num_swdge_queues=4)
    NW, G = cfg.nwin, cfg.group
    y = nc.dram_tensor("y_full", [cfg.N, 256], BF16, kind="ExternalInput")
    nbat = max(plan.nbatches, 1)
    gidx = nc.dram_tensor("gth_idx", list(plan.idx_wrapped.shape), I16,
                          kind="ExternalInput")
    rloc = nc.dram_tensor("row_local", [128, nbat], F32, kind="ExternalInput")
    iota_lo_d = nc.dram_tensor("iota_lo", [128, 128], BF16, kind="ExternalInput")
    iota_hi_d = nc.dram_tensor("iota_hi", [128, 128], BF16, kind="ExternalInput")
    x = nc.dram_tensor("x_own", [cfg.rpc, 128], F32, kind="ExternalInput")
    dis = nc.dram_tensor("dis_own", [cfg.rpc], F32, kind="ExternalInput")
    mu = nc.dram_tensor("mu", [1, 1], F32, kind="ExternalInput")
    p_in = nc.dram_tensor("p", [1, 1], F32, kind="ExternalInput")
    eps_in = nc.dram_tensor("eps", [1, 1], F32, kind="ExternalInput")
    if dynamic:
        rep_in = nc.dram_tensor("rep", [1, 1], I32, kind="ExternalInput")
    out = nc.dram_tensor("out_own", [cfg.rpc, 128], F32, kind="ExternalOutput")

    with tile.TileContext(nc) as tc, ExitStack() as ctx:
        cpool = ctx.enter_context(tc.tile_pool(name="c", bufs=1))
        stg = ctx.enter_context(tc.tile_pool(name="stg", bufs=3))
        ppool = ctx.enter_context(tc.tile_pool(name="ph", bufs=4))
        psum = ctx.enter_context(tc.tile_pool(name="ps", bufs=4, space="PSUM"))
        opool = ctx.enter_context(tc.tile_pool(name="op", bufs=3))

        # ---- constants / scalars ----
        idx_sb = cpool.tile(list(plan.idx_wrapped.shape), I16)
        nc.sync.dma_start(idx_sb[:], gidx.ap()[:])
        rl_sb = cpool.tile([128, nbat], F32)
        nc.sync.dma_start(rl_sb[:], rloc.ap()[:])
        io_lo = cpool.tile([128, 128], BF16)
        nc.sync.dma_start(io_lo[:], iota_lo_d.ap()[:])
        io_hi = cpool.tile([128, 128], BF16)
        nc.sync.dma_start(io_hi[:], iota_hi_d.ap()[:])
        dis_sb = cpool.tile([128, NW], F32)
        nc.sync.dma_start(dis_sb[:], dis.ap().rearrange("(w p) -> p w", p=128))

        psb = cpool.tile([1, 1], F32)
        nc.sync.dma_start(psb[:], p_in.ap()[:])
        sig = cpool.tile([1, 1], F32)
        nc.scalar.activation(sig[:], psb[:], ACT.Sigmoid)
        pp1 = cpool.tile([1, 1], F32)
        nc.vector.tensor_scalar(pp1[:], sig[:], 1.0, None, ALU.add)
        ipps = cpool.tile([1, 1], F32)
        nc.vector.reciprocal(ipps[:], pp1[:])
        ipp_vec = cpool.tile([128, 1], F32)
        nc.gpsimd.partition_broadcast(ipp_vec[:], ipps[:])

        esb = cpool.tile([1, 1], F32)
        nc.sync.dma_start(esb[:], eps_in.ap()[:])
        eb = cpool.tile([128, 1], F32)
        nc.gpsimd.partition_broadcast(eb[:], esb[:])
        oneps = cpool.tile([128, 1], F32)
        nc.vector.tensor_scalar(oneps[:], eb[:], 1.0, None, ALU.add)
        musb = cpool.tile([1, 1], F32)
        nc.sync.dma_start(musb[:], mu.ap()[:])
        mu_vec = cpool.tile([128, 1], F32)
        nc.gpsimd.partition_broadcast(mu_vec[:], musb[:])
        epsv = cpool.tile([128, 1], F32)
        nc.vector.memset(epsv[:], EPS_NUM)

        agg = cpool.tile([128, NW * 128], F32)

        loop_cm = ExitStack()
        if dynamic:
            rep_sb = cpool.tile([1, 1], I32)
            nc.sync.dma_start(rep_sb[:], rep_in.ap()[:])
            regs = []
            for e in mybir.ALL_ENGINES:
                regs.append(nc.alloc_register(e, f"repreg_{e.name}"))
            nc.regs_load(bass.RegisterHandles(tuple(regs)), rep_sb[0:1, 0:1])
            rep_val = bass.make_scalar_value(
                bass.RegisterHandles(tuple(regs)), min_val=0, max_val=1 << 20)
            loop_cm.enter_context(tc.For_i(0, rep_val, 1))

        with loop_cm:
            nc.vector.memset(agg[:], 0.0)

            yap = y.ap()
            live_psum = {}
            for ci, (bank, bt0, cn) in enumerate(plan.chunks):
                stage = stg.tile([128, cfg.chunk, 256], BF16)
                s0 = bt0 * 128
                nc.gpsimd.dma_gather(
                    stage[:, 0:cn, :],
                    yap[bank * cfg.bank_rows:
                        min((bank + 1) * cfg.bank_rows, cfg.N), :],
                    idx_sb[:, s0 // 16: (s0 + cn * 128) // 16],
                    cn * 128, cn * 128, 256, elem_step=256,
                    queue_num=ci % 4,
                )
                for k in range(cn):
                    bi = bt0 + k
                    bt = plan.batches[bi]
                    tgt = [("lo", bt.win)]
                    if bt.sec:
                        tgt.append(("hi", bt.win + 1))
                    for which, w in tgt:
                        key = (bt.bank, w // G)
                        if key not in live_psum:
                            live_psum[key] = psum.tile([128, G * 128], F32,
                                                       name="grp", tag="grp")
                        pt = live_psum[key]
                        P = ppool.tile([128, 128], BF16)
                        nc.vector.tensor_scalar(
                            P[:], io_lo[:] if which == "lo" else io_hi[:],
                            rl_sb[:, bi:bi + 1], None, ALU.is_equal)
                        first = plan.first_touch[(bt.bank, w)] == (bi, which)
                        last = plan.last_touch[(bt.bank, w)] == (bi, which)
                        sl = pt[:, (w % G) * 128:(w % G) * 128 + 128]
                        nc.tensor.matmul(sl, P[:], stage[:, k, 0:128],
                                         start=first, stop=False)
                        nc.tensor.matmul(sl, P[:], stage[:, k, 128:256],
                                         start=False, stop=last)
                    # flushes scheduled after this batch
                    for (fb, fg, wins) in plan.flushes.get(bi, []):
                        pt = live_psum.pop((fb, fg))
                        # contiguous runs of touched windows
                        runs = []
                        for w in wins:
                            if runs and w == runs[-1][1]:
                                runs[-1][1] = w + 1
                            else:
                                runs.append([w, w + 1])
                        for a, bnd in runs:
                            nc.vector.tensor_tensor(
                                agg[:, a * 128:bnd * 128],
                                agg[:, a * 128:bnd * 128],
                                pt[:, (a % G) * 128:(a % G) * 128 + (bnd - a) * 128],
                                ALU.add)

            # ---- output transform per window ----
            xt3 = x.ap().rearrange("(w p) f -> w p f", p=128)
            ot3 = out.ap().rearrange("(w p) f -> w p f", p=128)
            for w in range(NW):
                t1 = opool.tile([128, 128], F32)
                nc.scalar.activation(t1[:], agg[:, w * 128:(w + 1) * 128],
                                     ACT.Ln, bias=epsv[:, 0:1],
                                     scale=dis_sb[:, w:w + 1])
                t2 = opool.tile([128, 128], F32)
                nc.scalar.activation(t2[:], t1[:], ACT.Exp,
                                     scale=ipp_vec[:, 0:1])
                xt = opool.tile([128, 128], F32)
                nc.sync.dma_start(xt[:], xt3[w])
                xw = opool.tile([128, 128], F32)
                nc.scalar.activation(xw[:], xt[:], ACT.Identity,
                                     bias=mu_vec[:, 0:1], scale=oneps[:, 0:1])
                ot = opool.tile([128, 128], F32)
                nc.vector.tensor_tensor(ot[:], t2[:], xw[:], ALU.add)
                nc.sync.dma_start(ot3[w], ot[:])
    nc.compile()
    return nc


# ----------------------------------------------------------------------------
# PJRT runners
# ----------------------------------------------------------------------------

def _io_names(nc):
    in_names, out_names, out_avals = [], [], []
    import jax
    for alloc in nc.m.functions[0].allocations:
        if not isinstance(alloc, mybir.MemoryLocationSet):
            continue
        name = alloc.memorylocations[0].name
        if alloc.kind == "ExternalInput":
            if nc.partition_id_tensor is not None and \
                    name == nc.partition_id_tensor.name:
                continue
            in_names.append(name)
        elif alloc.kind == "ExternalOutput":
            out_names.append(name)
            out_avals.append(jax.core.ShapedArray(
                tuple(alloc.tensor_shape), mybir.dt.np(alloc.dtype)))
    return in_names, out_names, out_avals


def run_spmd(nc, in_maps):
    """Uniform program on len(in_maps) cores (the stock shard_map path)."""
    from concourse import bass2jax
    return bass2jax.run_bass_via_pjrt(nc, in_maps, n_cores=len(in_maps))


class SingleRunner:
    """One program pinned to one device; supports async dispatch."""

    def __init__(self, nc, device):
        import jax
        from concourse.bass2jax import _bass_exec_p, install_neuronx_cc_hook
        install_neuronx_cc_hook()
        assert nc.partition_id_tensor is None, "per-core programs must not use partition id"
        self.nc, self.device = nc, device
        self.in_names, self.out_names, self.out_avals = _io_names(nc)
        all_in = tuple(self.in_names + self.out_names)
        out_avals = tuple(self.out_avals)
        out_names = tuple(self.out_names)

        def _body(*args):
            outs = _bass_exec_p.bind(
                *args, out_avals=out_avals, in_names=all_in,
                out_names=out_names, lowering_input_output_aliases=(),
                sim_require_finite=True, sim_require_nnan=True, nc=nc)
            return tuple(outs)

        n_params = len(self.in_names)
        donate = tuple(range(n_params, n_params + len(out_names)))
        self.fn = jax.jit(_body, donate_argnums=donate, keep_unused=True)
        self._dev_inputs = None

    def put_inputs(self, in_map):
        import jax
        self._dev_inputs = [jax.device_put(np.asarray(in_map[n]), self.device)
                            for n in self.in_names]
        jax.block_until_ready(self._dev_inputs)

    def dispatch(self):
        import jax
        import jax.numpy as jnp
        zeros = [jnp.zeros(a.shape, a.dtype, device=self.device)
                 for a in self.out_avals]
        return self.fn(*self._dev_inputs, *zeros)

    def collect(self, futs):
        return {n: np.asarray(f) for n, f in zip(self.out_names, futs)}


# ----------------------------------------------------------------------------
# numpy emulation of the planned P2 schedule (host-side logic check only)
# ----------------------------------------------------------------------------

def emulate_p2(cfg, plan, y_full_pair, x_own, dis_own, mu, pp, eps):
    yhi = y_full_pair[:, 0:128].astype(np.float32)
    ylo = y_full_pair[:, 128:256].astype(np.float32)
    agg = np.zeros((128, cfg.nwin * 128), np.float32)
    for bi, bt in enumerate(plan.batches):
        s0 = bi * 128
        idx = np.zeros(128, np.int64)
        for i in range(128):
            idx[i] = plan.idx_wrapped[(s0 + i) % 16, (s0 + i) // 16]
        gl = bt.bank * cfg.bank_rows + idx
        rl = plan.row_local[:, bi].astype(np.float32)
        for which, w in ([("lo", bt.win)] + ([("hi", bt.win + 1)] if bt.sec else [])):
            base = 0.0 if which == "lo" else 128.0
            P = (rl[:, None] == (np.arange(128)[None, :] + base))
            v = P.T.astype(np.float32) @ (yhi[gl] + ylo[gl])
            agg[:, w * 128:(w + 1) * 128] += v
    # output transform
    nodes = (np.arange(cfg.nwin * 128) % 128)[None]
    aggn = np.zeros((cfg.rpc, 128), np.float32)
    for w in range(cfg.nwin):
        aggn[w * 128:(w + 1) * 128, :] = agg[:, w * 128:(w + 1) * 128]
    o = np.exp((1.0 / pp) * np.log(dis_own[:, None] * aggn + EPS_NUM))
    return o + (1 + eps) * x_own + mu


# ----------------------------------------------------------------------------
# public entry
# ----------------------------------------------------------------------------

_CACHE = {}


def _setup_jax():
    import jax
    cache = "/tmp/jax_neff_cache"
    os.makedirs(cache, exist_ok=True)
    try:
        jax.config.update("jax_compilation_cache_dir", cache)
        jax.config.update("jax_persistent_cache_min_entry_size_bytes", -1)
        jax.config.update("jax_persistent_cache_min_compile_time_secs", 0.0)
    except Exception:
        pass


def _pad_rows(a, rows, fill):
    if a.shape[0] == rows:
        return np.ascontiguousarray(a)
    out = np.full((rows,) + a.shape[1:], fill, a.dtype)
    out[: a.shape[0]] = a
    return out


def kernel(x, eps, p, edge_index):
    import jax
    _setup_jax()
    cfg = Cfg()
    x = np.asarray(x, np.float32)
    eps = np.asarray(eps, np.float32).reshape(1, 1)
    p = np.asarray(p, np.float32).reshape(1, 1)
    edge_index = np.asarray(edge_index)
    assert x.shape == (cfg.N, 128)

    p0_plans, plans = plan_all(edge_index, cfg)
    x_sl = [
        _pad_rows(x[c * cfg.rpc_real:(c + 1) * cfg.rpc_real], cfg.rpc, 1e30)
        for c in range(cfg.ncores)
    ]
    io_lo, io_hi = _iota_tiles()
    devices = jax.devices()[: cfg.ncores]

    # ---- P0 (per-core programs, concurrent) ----
    runners0 = []
    for c in range(cfg.ncores):
        key0 = ("p0", cfg.N, cfg.E, c,
                hash(p0_plans[c].row_local.tobytes()))
        if key0 not in _CACHE:
            _CACHE[key0] = build_p0(cfg, p0_plans[c])
        runners0.append(SingleRunner(_CACHE[key0], devices[c]))
    for c in range(cfg.ncores):
        runners0[c].put_inputs({
            "x_own": x_sl[c], "col_local": p0_plans[c].row_local,
            "iota_lo": io_lo, "iota_hi": io_hi,
        })
    futs0 = [r.dispatch() for r in runners0]
    jax.block_until_ready(futs0)
    res0 = [runners0[c].collect(futs0[c]) for c in range(cfg.ncores)]
    mu = np.array(min(float(r["xmin"][0, 0]) for r in res0), np.float32)
    mu = mu.reshape(1, 1)
    dis_sl = [res0[c]["dis_own"] for c in range(cfg.ncores)]

    # ---- P1 ----
    key1 = ("p1", cfg.N)
    if key1 not in _CACHE:
        _CACHE[key1] = build_p1(cfg)
    nc1 = _CACHE[key1]
    in_maps1 = [
        {"x_own": x_sl[c], "dis_own": dis_sl[c], "mu": mu, "p": p}
        for c in range(cfg.ncores)
    ]
    res1 = run_spmd(nc1, in_maps1)
    y_full = np.concatenate(
        [res1[c]["y_own"][: cfg.rpc_real] for c in range(cfg.ncores)], axis=0)

    # ---- P2 ----
    outs = [None] * cfg.ncores
    runners = []
    for c in range(cfg.ncores):
        key2 = ("p2", cfg.N, cfg.E, c,
                hash(plans[c].idx_wrapped.tobytes()),
                hash(plans[c].row_local.tobytes()))
        if key2 not in _CACHE:
            _CACHE[key2] = build_p2(cfg, plans[c])
        runners.append(SingleRunner(_CACHE[key2], devices[c]))
    for c in range(cfg.ncores):
        runners[c].put_inputs({
            "y_full": y_full, "gth_idx": plans[c].idx_wrapped,
            "row_local": plans[c].row_local, "iota_lo": io_lo,
            "iota_hi": io_hi, "x_own": x_sl[c], "dis_own": dis_sl[c],
            "mu": mu, "p": p, "eps": eps,
        })
    futs = [runners[c].dispatch() for c in range(cfg.ncores)]
    jax.block_until_ready(futs)
    for c in range(cfg.ncores):
        outs[c] = runners[c].collect(futs[c])["out_own"][: cfg.rpc_real]
    return np.concatenate(outs, axis=0)



# revision 8
# speedup vs baseline: 2.2371x; 2.2371x over previous
"""GCNConv-variant Trainium2 kernel (8 NeuronCores, SPMD via bass/tile).

Math (from the reference):
    deg  = in-degree of col over all edges               [N]
    dis  = where(deg>0, deg^-1/2, 0)                     [N]
    pp   = sigmoid(p) + 1
    mu   = min(x)
    y    = dis * (x - mu + 1e-6)^pp                      [N,128]  (bf16)
    agg[i] = sum_{e: row[e]==i} y[col[e]]
    out  = (dis*agg + 1e-6)^(1/pp) + (1+eps)*x + mu

Distribution (3 launches, node ranges owned per core):
    L0 (uniform SPMD): per-core x-slice min -> host min -> mu.
    L1 (uniform SPMD): per-core deg (diff of host searchsorted offsets of
       its sorted owned-col list), dis, y for the owned range, y in bf16.
    L2 (one program per core): gather y rows by col (SWDGE dma_gather,
       single_packet=False, 4 queues, deep stage buffering); one-hot bf16
       matmuls (P^T @ y_batch) accumulate window segment-sums in PSUM;
       output transform reads PSUM directly (no SBUF accumulator) since
       edges are sorted group-major (grp, bank, row).
"""

import os
import sys
from contextlib import ExitStack

sys.path.insert(0, "/opt/trn_rl_repo")

import numpy as np
import ml_dtypes

import concourse.bass as bass
import concourse.bacc as bacc
import concourse.bass_isa as bass_isa
import concourse.mybir as mybir
import concourse.tile as tile

F32 = mybir.dt.float32
BF16 = mybir.dt.bfloat16
I16 = mybir.dt.int16
I32 = mybir.dt.int32
EPS_NUM = 1e-6
ALU = mybir.AluOpType
ACT = mybir.ActivationFunctionType


class Cfg:
    def __init__(self, N=100000, E=1600000, D=128, ncores=8, bank_rows=25000,
                 chunk=8, group=4, stage_bufs=10):
        assert D == 128
        self.N, self.E, self.D, self.ncores = N, E, D, ncores
        self.rpc_real = N // ncores            # owned rows per core
        assert self.rpc_real * ncores == N
        self.rpc = ((self.rpc_real + 127) // 128) * 128   # padded rows
        self.nwin = self.rpc // 128            # row windows per core
        self.bank_rows = bank_rows             # gather bank size (int16 limit)
        assert bank_rows <= 32768
        self.nbanks = (N + bank_rows - 1) // bank_rows
        self.chunk = chunk                     # gather batches per SWDGE call
        self.group = group                     # windows per PSUM bank tile
        self.ngrp = (self.nwin + group - 1) // group
        self.stage_bufs = stage_bufs


# ----------------------------------------------------------------------------
# host-side planning (pure index/layout work on edge_index; no float math)
# ----------------------------------------------------------------------------

def _wrap_idxs(idx_linear):
    """SWDGE index layout: slot i lives at [i%16, i//16], tiled to 128 parts."""
    n = len(idx_linear)
    assert n % 16 == 0
    a = np.zeros((16, n // 16), np.int16)
    ar = np.arange(n)
    a[ar % 16, ar // 16] = idx_linear.astype(np.int16)
    return np.tile(a, (8, 1))


class CorePlan:
    pass


def plan_core_p2(rows_local, cols, cfg: Cfg):
    """Plan one core's L2 schedule. rows_local in [0, rpc_real).

    Sort edges (bank, row); batches of <=128 edges within one (bank, win)
    so every batch targets exactly one window (no straddle) and PSUM
    accumulation groups within a bank tile open/close strictly
    sequentially. PSUM tiles keyed (bank, grp); flushed by DVE add into
    the SBUF agg accumulator; transform per grp follows the last flush
    that touches it.
    """
    G = cfg.group
    win = rows_local // 128
    grp = win // G
    bank = cols // cfg.bank_rows
    order = np.lexsort((rows_local, bank))
    r = rows_local[order]
    w = win[order]
    g = grp[order]
    b = bank[order]
    c = (cols - bank * cfg.bank_rows)[order]

    n = len(r)
    batches = []     # (grp, bank, win)
    rls = []
    idx_parts = []
    i = 0
    while i < n:
        j = min(i + 128, n)
        cut = j - i
        for k in range(i + 1, j):
            if w[k] != w[i] or b[k] != b[i]:
                cut = k - i
                break
        j = i + cut
        take = j - i
        rl = np.concatenate([r[i:j] - int(w[i]) * 128,
                             np.full(128 - take, -1, np.int64)])
        idx = np.concatenate([c[i:j], np.zeros(128 - take, np.int64)])
        batches.append((int(g[i]), int(b[i]), int(w[i])))
        rls.append(rl.astype(np.float32))
        idx_parts.append(idx)
        i = j

    nb = len(batches)
    plan = CorePlan()
    plan.nbatches = nb
    plan.batches = batches
    if nb == 0:
        plan.idx_wrapped = np.zeros((128, 8), np.int16)
        plan.row_local = np.zeros((128, 1), np.float32)
        plan.chunks = []
        plan.first_touch = {}
        plan.last_touch = {}
        plan.flushes = {}
        plan.transforms = {}
        return plan
    plan.idx_wrapped = _wrap_idxs(np.concatenate(idx_parts))
    plan.row_local = np.stack(rls, axis=1)

    # gather chunks: runs of batches with same bank, up to cfg.chunk
    chunks = []   # (bank, bt0, nbatch)
    i = 0
    while i < nb:
        bb = batches[i][1]
        j = i
        while j < nb and batches[j][1] == bb and j - i < cfg.chunk:
            j += 1
        chunks.append((bb, i, j - i))
        i = j
    plan.chunks = chunks

    # per-(bank, win) first/last batch index -> matmul start/stop
    first_touch = {}
    last_touch = {}
    for bi, (gg, bb, ww) in enumerate(batches):
        key = (bb, ww)
        if key not in first_touch:
            first_touch[key] = bi
        last_touch[key] = bi
    plan.first_touch = first_touch
    plan.last_touch = last_touch

    # flushes: batch idx -> list of (bank, grp, [touched wins sorted])
    bg_last = {}
    bg_wins = {}
    for bi, (gg, bb, ww) in enumerate(batches):
        bg_last[(bb, gg)] = bi
        bg_wins.setdefault((bb, gg), set()).add(ww)
    flushes = {}
    for (bb, gg), last_bi in bg_last.items():
        flushes.setdefault(last_bi, []).append(
            (bb, gg, sorted(bg_wins[(bb, gg)])))
    plan.flushes = flushes

    # transforms: batch idx -> list of grps fully flushed after that batch
    grp_done = {}
    for (bb, gg), last_bi in bg_last.items():
        grp_done[gg] = max(grp_done.get(gg, -1), last_bi)
    transforms = {}
    for gg, bi in grp_done.items():
        transforms.setdefault(bi, []).append(gg)
    plan.transforms = transforms
    plan.grps_with_batches = set(grp_done.keys())
    return plan


def plan_all(edge_index, cfg: Cfg):
    """Shard edges. Returns (offsets per core, p2 plans per core)."""
    row = np.asarray(edge_index[0]).astype(np.int64)
    col = np.asarray(edge_index[1]).astype(np.int64)

    # ---- L1: per-core owned-col offsets (host indexing only) ----
    owner = col // cfg.rpc_real
    offs = []
    for cr in range(cfg.ncores):
        loc = np.sort(col[owner == cr] - cr * cfg.rpc_real)
        off = np.searchsorted(loc, np.arange(cfg.rpc + 1)).astype(np.float32)
        offs.append(off)

    # ---- L2: row shards ----
    rowner = row // cfg.rpc_real
    plans = []
    for cr in range(cfg.ncores):
        sel = rowner == cr
        plans.append(plan_core_p2((row[sel] - cr * cfg.rpc_real), col[sel], cfg))
    return offs, plans


# ----------------------------------------------------------------------------
# bass program builders
# ----------------------------------------------------------------------------

def _patch_act_tables(arch):
    """Steer the act-table chooser to the combined Ln+Exp set.

    The insert_act_table_loads pass picks the first act_info.json set
    containing each required function, which ping-pongs between the
    Ln-only and Exp-only sets (a ~1.3us table DMA per switch). Ln and Exp
    coexist in natural_log_exp_and_others; pruning them from the other
    sets (in the cached dict, preserving set ids) makes the chooser pick
    the combined set once. The emitted loads stay semantically correct —
    the chosen table genuinely contains every function used under it.
    """
    from concourse.hw_specs import get_activation_tables
    t = get_activation_tables(arch)
    combined = "natural_log_exp_and_others"
    if combined not in t:
        return
    for name, s in t.items():
        if name != combined:
            s.discard(ACT.Ln)
            s.discard(ACT.Exp)


def _mk_nc(**kw):
    nc = bacc.Bacc("TRN2", target_bir_lowering=False, debug=False,
                   enable_partition_id=False, **kw)
    _patch_act_tables(nc.m.arch)
    return nc


def _iota_tile():
    i = np.arange(128, dtype=np.float32)
    return np.tile(i, (128, 1)).astype(ml_dtypes.bfloat16)


def _dyn_loop(nc, tc, cpool, ctx, dynamic):
    if not dynamic:
        return
    rep_in = nc.dram_tensor("rep", [1, 1], I32, kind="ExternalInput")
    rep_sb = cpool.tile([1, 1], I32)
    nc.sync.dma_start(rep_sb[:], rep_in.ap()[:])
    regs = []
    for e in mybir.ALL_ENGINES:
        regs.append(nc.alloc_register(e, f"repreg_{e.name}"))
    nc.regs_load(bass.RegisterHandles(tuple(regs)), rep_sb[0:1, 0:1])
    rep_val = bass.make_scalar_value(
        bass.RegisterHandles(tuple(regs)), min_val=0, max_val=1 << 20)
    ctx.enter_context(tc.For_i(0, rep_val, 1))


def build_min(cfg: Cfg, rep=0):
    """Uniform SPMD program: x-slice min."""
    dynamic = rep == -1
    nc = _mk_nc()
    x = nc.dram_tensor("x_own", [cfg.rpc, 128], F32, kind="ExternalInput")
    xmin = nc.dram_tensor("xmin", [1, 1], F32, kind="ExternalOutput")
    NW = cfg.nwin
    with tile.TileContext(nc) as tc, ExitStack() as ctx:
        pool = ctx.enter_context(tc.tile_pool(name="m", bufs=3))
        cpool = ctx.enter_context(tc.tile_pool(name="mc", bufs=1))
        runmin = cpool.tile([128, 1], F32)
        _dyn_loop(nc, tc, cpool, ctx, dynamic)
        SW = 4
        nt4 = (NW // SW) * SW * 128
        xt4 = x.ap()[0:nt4, :].rearrange("(t w p) f -> t p w f", p=128, w=SW)
        xt3 = x.ap().rearrange("(w p) f -> w p f", p=128)
        nc.vector.memset(runmin[:], 1e30)
        for t in range(NW // SW):
            xt = pool.tile([128, SW, 128], F32)
            nc.sync.dma_start(xt[:], xt4[t])
            red = pool.tile([128, 1], F32)
            nc.vector.tensor_reduce(red[:], xt[:], mybir.AxisListType.XY, ALU.min)
            nc.vector.tensor_tensor(runmin[:], runmin[:], red[:], ALU.min)
        for w in range((NW // SW) * SW, NW):
            xt1 = pool.tile([128, 128], F32, name="xt1", tag="xt1")
            nc.sync.dma_start(xt1[:], xt3[w])
            red1 = pool.tile([128, 1], F32, name="red1", tag="red1")
            nc.vector.tensor_reduce(red1[:], xt1[:], mybir.AxisListType.X, ALU.min)
            nc.vector.tensor_tensor(runmin[:], runmin[:], red1[:], ALU.min)
        negmin = cpool.tile([128, 1], F32, name="negmin", tag="negmin")
        nc.vector.tensor_scalar(negmin[:], runmin[:], -1.0, None, ALU.mult)
        allmax = cpool.tile([128, 1], F32, name="allmax", tag="allmax")
        nc.gpsimd.partition_all_reduce(allmax[:], negmin[:], 128,
                                       bass_isa.ReduceOp.max)
        minv = cpool.tile([1, 1], F32, name="minv", tag="minv")
        nc.vector.tensor_scalar(minv[:], allmax[0:1, :], -1.0, None, ALU.mult)
        nc.sync.dma_start(xmin.ap()[:], minv[:])
    nc.compile()
    return nc


def build_y(cfg: Cfg, rep=0):
    """Uniform SPMD program: deg -> dis -> y (bf16) for the owned slice."""
    dynamic = rep == -1
    nc = _mk_nc()
    x = nc.dram_tensor("x_own", [cfg.rpc, 128], F32, kind="ExternalInput")
    off = nc.dram_tensor("off_own", [cfg.rpc + 128], F32, kind="ExternalInput")
    mu = nc.dram_tensor("mu", [1, 1], F32, kind="ExternalInput")
    p_in = nc.dram_tensor("p", [1, 1], F32, kind="ExternalInput")
    y = nc.dram_tensor("y_own", [cfg.rpc, 128], BF16, kind="ExternalOutput")
    dis_out = nc.dram_tensor("dis_own", [cfg.rpc], F32, kind="ExternalOutput")

    NW = cfg.nwin
    with tile.TileContext(nc) as tc, ExitStack() as ctx:
        pool = ctx.enter_context(tc.tile_pool(name="y", bufs=3))
        cpool = ctx.enter_context(tc.tile_pool(name="yc", bufs=1))

        # scalars
        psb = cpool.tile([1, 1], F32)
        nc.sync.dma_start(psb[:], p_in.ap()[:])
        sig = cpool.tile([1, 1], F32)
        nc.scalar.activation(sig[:], psb[:], ACT.Sigmoid)
        ppb = cpool.tile([128, 1], F32)
        nc.gpsimd.partition_broadcast(ppb[:], sig[:])
        pp_vec = cpool.tile([128, 1], F32)
        nc.vector.tensor_scalar(pp_vec[:], ppb[:], 1.0, None, ALU.add)
        musb = cpool.tile([1, 1], F32)
        nc.sync.dma_start(musb[:], mu.ap()[:])
        mub = cpool.tile([128, 1], F32)
        nc.gpsimd.partition_broadcast(mub[:], musb[:])
        cvec = cpool.tile([128, 1], F32)   # 1e-6 - mu
        nc.vector.tensor_scalar(cvec[:], mub[:], -1.0, EPS_NUM, ALU.mult, ALU.add)

        # ---- deg = off[n+1] - off[n]; ldis = -0.5*ln(max(deg,1)) - 100*(deg==0)
        offA = cpool.tile([128, NW], F32)
        nc.sync.dma_start(offA[:], off.ap()[1:cfg.rpc + 1]
                          .rearrange("(w p) -> p w", p=128))
        offB = cpool.tile([128, NW], F32)
        nc.sync.dma_start(offB[:], off.ap()[0:cfg.rpc]
                          .rearrange("(w p) -> p w", p=128))
        deg = cpool.tile([128, NW], F32)
        nc.vector.tensor_tensor(deg[:], offA[:], offB[:], ALU.subtract)
        mask = cpool.tile([128, NW], F32)
        nc.vector.tensor_scalar(mask[:], deg[:], 0.5, None, ALU.is_ge)
        degc = cpool.tile([128, NW], F32)
        nc.vector.tensor_scalar(degc[:], deg[:], 1.0, None, ALU.max)
        ldeg = cpool.tile([128, NW], F32)
        nc.scalar.activation(ldeg[:], degc[:], ACT.Ln)
        ldis = cpool.tile([128, NW], F32)
        nc.vector.tensor_scalar(ldis[:], ldeg[:], -0.5, None, ALU.mult)
        pen = cpool.tile([128, NW], F32)
        nc.vector.tensor_scalar(pen[:], mask[:], 100.0, -100.0, ALU.mult, ALU.add)
        nc.vector.tensor_tensor(ldis[:], ldis[:], pen[:], ALU.add)
        dis = cpool.tile([128, NW], F32)
        nc.scalar.activation(dis[:], ldis[:], ACT.Exp)
        nc.sync.dma_start(dis_out.ap().rearrange("(w p) -> p w", p=128), dis[:])

        _dyn_loop(nc, tc, cpool, ctx, dynamic)
        # y = exp(pp*ln(x - mu + eps) + ldis), 4-window supertiles
        SW = 4
        nt4 = (NW // SW) * SW * 128
        xt4 = x.ap()[0:nt4, :].rearrange("(t w p) f -> t p w f", p=128, w=SW)
        yt4 = y.ap()[0:nt4, :].rearrange("(t w p) f -> t p w f", p=128, w=SW)
        for t in range(NW // SW):
            xt = pool.tile([128, SW, 128], F32)
            nc.sync.dma_start(xt[:], xt4[t])
            t1 = pool.tile([128, SW, 128], F32)
            nc.scalar.activation(t1[:], xt[:], ACT.Ln, bias=cvec[:, 0:1])
            yt = pool.tile([128, SW, 128], BF16)
            for w in range(SW):
                nc.scalar.activation(yt[:, w, :], t1[:, w, :], ACT.Exp,
                                     bias=ldis[:, t * SW + w:t * SW + w + 1],
                                     scale=pp_vec[:, 0:1])
            nc.sync.dma_start(yt4[t], yt[:])
        xt3 = x.ap().rearrange("(w p) f -> w p f", p=128)
        yt3 = y.ap().rearrange("(w p) f -> w p f", p=128)
        for w in range((NW // SW) * SW, NW):
            xt1 = pool.tile([128, 128], F32, name="xt1", tag="xt1")
            nc.sync.dma_start(xt1[:], xt3[w])
            t1b = pool.tile([128, 128], F32, name="t1b", tag="t1b")
            nc.scalar.activation(t1b[:], xt1[:], ACT.Ln, bias=cvec[:, 0:1])
            ytb = pool.tile([128, 128], BF16, name="ytb", tag="ytb")
            nc.scalar.activation(ytb[:], t1b[:], ACT.Exp,
                                 bias=ldis[:, w:w + 1], scale=pp_vec[:, 0:1])
            nc.sync.dma_start(yt3[w], ytb[:])
    nc.compile()
    return nc


def build_p2(cfg: Cfg, plan: CorePlan, rep=0):
    """Per-core program: gather + one-hot matmul segment-sum + transform."""
    dynamic = rep == -1
    nc = _mk_nc(num_swdge_queues=4)
    NW, G = cfg.nwin, cfg.group
    y = nc.dram_tensor("y_full", [cfg.N, 128], BF16, kind="ExternalInput")
    nbat = max(plan.nbatches, 1)
    gidx = nc.dram_tensor("gth_idx", list(plan.idx_wrapped.shape), I16,
                          kind="ExternalInput")
    rloc = nc.dram_tensor("row_local", [128, nbat], F32, kind="ExternalInput")
    iota_d = nc.dram_tensor("iota", [128, 128], BF16, kind="ExternalInput")
    x = nc.dram_tensor("x_own", [cfg.rpc, 128], F32, kind="ExternalInput")
    dis = nc.dram_tensor("dis_own", [cfg.rpc], F32, kind="ExternalInput")
    mu = nc.dram_tensor("mu", [1, 1], F32, kind="ExternalInput")
    p_in = nc.dram_tensor("p", [1, 1], F32, kind="ExternalInput")
    eps_in = nc.dram_tensor("eps", [1, 1], F32, kind="ExternalInput")
    out = nc.dram_tensor("out_own", [cfg.rpc, 128], F32, kind="ExternalOutput")

    with tile.TileContext(nc) as tc, ExitStack() as ctx:
        cpool = ctx.enter_context(tc.tile_pool(name="c", bufs=1))
        stg = ctx.enter_context(tc.tile_pool(name="stg", bufs=cfg.stage_bufs))
        ppool = ctx.enter_context(tc.tile_pool(name="ph", bufs=6))
        psum = ctx.enter_context(tc.tile_pool(name="ps", bufs=4, space="PSUM"))
        opool = ctx.enter_context(tc.tile_pool(name="op", bufs=4))

        # ---- constants / scalars ----
        idx_sb = cpool.tile(list(plan.idx_wrapped.shape), I16)
        nc.sync.dma_start(idx_sb[:], gidx.ap()[:])
        rl_sb = cpool.tile([128, nbat], F32)
        nc.sync.dma_start(rl_sb[:], rloc.ap()[:])
        io_sb = cpool.tile([128, 128], BF16)
        nc.sync.dma_start(io_sb[:], iota_d.ap()[:])
        dis_sb = cpool.tile([128, NW], F32)
        nc.sync.dma_start(dis_sb[:], dis.ap().rearrange("(w p) -> p w", p=128))

        psb = cpool.tile([1, 1], F32)
        nc.sync.dma_start(psb[:], p_in.ap()[:])
        sig = cpool.tile([1, 1], F32)
        nc.scalar.activation(sig[:], psb[:], ACT.Sigmoid)
        pp1 = cpool.tile([1, 1], F32)
        nc.vector.tensor_scalar(pp1[:], sig[:], 1.0, None, ALU.add)
        ipps = cpool.tile([1, 1], F32)
        nc.vector.reciprocal(ipps[:], pp1[:])
        ipp_vec = cpool.tile([128, 1], F32)
        nc.gpsimd.partition_broadcast(ipp_vec[:], ipps[:])

        esb = cpool.tile([1, 1], F32)
        nc.sync.dma_start(esb[:], eps_in.ap()[:])
        eb = cpool.tile([128, 1], F32)
        nc.gpsimd.partition_broadcast(eb[:], esb[:])
        oneps = cpool.tile([128, 1], F32)
        nc.vector.tensor_scalar(oneps[:], eb[:], 1.0, None, ALU.add)
        musb = cpool.tile([1, 1], F32)
        nc.sync.dma_start(musb[:], mu.ap()[:])
        mu_vec = cpool.tile([128, 1], F32)
        nc.gpsimd.partition_broadcast(mu_vec[:], musb[:])
        epsv = cpool.tile([128, 1], F32)
        nc.vector.memset(epsv[:], EPS_NUM)

        loop_cm = ExitStack()
        if dynamic:
            rep_in = nc.dram_tensor("rep", [1, 1], I32, kind="ExternalInput")
            rep_sb = cpool.tile([1, 1], I32)
            nc.sync.dma_start(rep_sb[:], rep_in.ap()[:])
            regs = []
            for e in mybir.ALL_ENGINES:
                regs.append(nc.alloc_register(e, f"repreg_{e.name}"))
            nc.regs_load(bass.RegisterHandles(tuple(regs)), rep_sb[0:1, 0:1])
            rep_val = bass.make_scalar_value(
                bass.RegisterHandles(tuple(regs)), min_val=0, max_val=1 << 20)
            loop_cm.enter_context(tc.For_i(0, rep_val, 1))

        xt3 = x.ap().rearrange("(w p) f -> w p f", p=128)
        ot3 = out.ap().rearrange("(w p) f -> w p f", p=128)
        agg = cpool.tile([128, NW * 128], F32)

        def transform_grp(gg):
            """Output transform for grp gg, reading the SBUF agg slice."""
            w0 = gg * G
            for w in range(w0, min(w0 + G, NW)):
                t1 = opool.tile([128, 128], F32, name="t1", tag="t1")
                nc.scalar.activation(t1[:], agg[:, w * 128:(w + 1) * 128],
                                     ACT.Ln, bias=epsv[:, 0:1],
                                     scale=dis_sb[:, w:w + 1])
                t2 = opool.tile([128, 128], F32, name="t2", tag="t2")
                nc.scalar.activation(t2[:], t1[:], ACT.Exp,
                                     scale=ipp_vec[:, 0:1])
                xt = opool.tile([128, 128], F32, name="xt", tag="xt")
                nc.sync.dma_start(xt[:], xt3[w])
                xw = opool.tile([128, 128], F32, name="xw", tag="xw")
                nc.scalar.activation(xw[:], xt[:], ACT.Identity,
                                     bias=mu_vec[:, 0:1], scale=oneps[:, 0:1])
                ot = opool.tile([128, 128], F32, name="ot", tag="ot")
                nc.vector.tensor_tensor(ot[:], t2[:], xw[:], ALU.add)
                nc.sync.dma_start(ot3[w], ot[:])

        with loop_cm:
            nc.vector.memset(agg[:], 0.0)
            yap = y.ap()
            live_psum = {}
            for ci, (bank, bt0, cn) in enumerate(plan.chunks):
                stage = stg.tile([128, cfg.chunk, 128], BF16)
                s0 = bt0 * 128
                nc.gpsimd.dma_gather(
                    stage[:, 0:cn, :],
                    yap[bank * cfg.bank_rows:
                        min((bank + 1) * cfg.bank_rows, cfg.N), :],
                    idx_sb[:, s0 // 16: (s0 + cn * 128) // 16],
                    cn * 128, cn * 128, 128, elem_step=128,
                    queue_num=ci % 4, single_packet=False,
                )
                for k in range(cn):
                    bi = bt0 + k
                    gg, bb, w = plan.batches[bi]
                    key = (bb, gg)
                    if key not in live_psum:
                        live_psum[key] = psum.tile([128, G * 128], F32,
                                                   name="grp", tag="grp")
                    pt = live_psum[key]
                    P = ppool.tile([128, 128], BF16)
                    nc.vector.tensor_scalar(P[:], io_sb[:],
                                            rl_sb[:, bi:bi + 1], None,
                                            ALU.is_equal)
                    first = plan.first_touch[(bb, w)] == bi
                    last = plan.last_touch[(bb, w)] == bi
                    sl = pt[:, (w % G) * 128:(w % G) * 128 + 128]
                    nc.tensor.matmul(sl, P[:], stage[:, k, :],
                                     start=first, stop=last)
                    # flushes + transforms scheduled after this batch
                    for (fb, fg, wins) in plan.flushes.get(bi, []):
                        pt2 = live_psum.pop((fb, fg))
                        runs = []
                        for ww in wins:
                            if runs and ww == runs[-1][1]:
                                runs[-1][1] = ww + 1
                            else:
                                runs.append([ww, ww + 1])
                        for a, bnd in runs:
                            nc.vector.tensor_tensor(
                                agg[:, a * 128:bnd * 128],
                                agg[:, a * 128:bnd * 128],
                                pt2[:, (a % G) * 128:
                                    (a % G) * 128 + (bnd - a) * 128],
                                ALU.add)
                    for gg2 in plan.transforms.get(bi, []):
                        transform_grp(gg2)
            # grps with no batches at all still need output
            for gg in range(cfg.ngrp):
                if gg not in plan.grps_with_batches:
                    transform_grp(gg)
    nc.compile()
    return nc


# ----------------------------------------------------------------------------
# PJRT runners
# ----------------------------------------------------------------------------

def _io_names(nc):
    in_names, out_names, out_avals = [], [], []
    import jax
    for alloc in nc.m.functions[0].allocations:
        if not isinstance(alloc, mybir.MemoryLocationSet):
            continue
        name = alloc.memorylocations[0].name
        if alloc.kind == "ExternalInput":
            if nc.partition_id_tensor is not None and \
                    name == nc.partition_id_tensor.name:
                continue
            in_names.append(name)
        elif alloc.kind == "ExternalOutput":
            out_names.append(name)
            out_avals.append(jax.core.ShapedArray(
                tuple(alloc.tensor_shape), mybir.dt.np(alloc.dtype)))
    return in_names, out_names, out_avals


def run_spmd(nc, in_maps):
    """Uniform program on len(in_maps) cores (the stock shard_map path)."""
    from concourse import bass2jax
    return bass2jax.run_bass_via_pjrt(nc, in_maps, n_cores=len(in_maps))


class SingleRunner:
    """One program pinned to one device; supports async dispatch."""

    def __init__(self, nc, device):
        import jax
        from concourse.bass2jax import _bass_exec_p, install_neuronx_cc_hook
        install_neuronx_cc_hook()
        assert nc.partition_id_tensor is None, \
            "per-core programs must not use partition id"
        self.nc, self.device = nc, device
        self.in_names, self.out_names, self.out_avals = _io_names(nc)
        all_in = tuple(self.in_names + self.out_names)
        out_avals = tuple(self.out_avals)
        out_names = tuple(self.out_names)

        def _body(*args):
            outs = _bass_exec_p.bind(
                *args, out_avals=out_avals, in_names=all_in,
                out_names=out_names, lowering_input_output_aliases=(),
                sim_require_finite=True, sim_require_nnan=True, nc=nc)
            return tuple(outs)

        n_params = len(self.in_names)
        donate = tuple(range(n_params, n_params + len(out_names)))
        self.fn = jax.jit(_body, donate_argnums=donate, keep_unused=True)
        self._dev_inputs = None

    def put_inputs(self, in_map):
        import jax
        self._dev_inputs = [jax.device_put(np.asarray(in_map[n]), self.device)
                            for n in self.in_names]
        jax.block_until_ready(self._dev_inputs)

    def dispatch(self):
        import jax
        import jax.numpy as jnp
        zeros = [jnp.zeros(a.shape, a.dtype, device=self.device)
                 for a in self.out_avals]
        return self.fn(*self._dev_inputs, *zeros)

    def collect(self, futs):
        return {n: np.asarray(f) for n, f in zip(self.out_names, futs)}


# ----------------------------------------------------------------------------
# numpy emulation of the planned L2 schedule (host-side logic check only)
# ----------------------------------------------------------------------------

def emulate_p2(cfg, plan, y_full, x_own, dis_own, mu, pp, eps):
    yf = np.asarray(y_full).astype(np.float32)
    agg = np.zeros((cfg.rpc, 128), np.float32)
    for bi, (gg, bb, w) in enumerate(plan.batches):
        s0 = bi * 128
        idx = np.zeros(128, np.int64)
        for i in range(128):
            idx[i] = plan.idx_wrapped[(s0 + i) % 16, (s0 + i) // 16]
        gl = bb * cfg.bank_rows + idx
        rl = plan.row_local[:, bi]
        Pm = (rl[:, None] == np.arange(128)[None, :]).astype(np.float32)
        agg[w * 128:(w + 1) * 128, :] += Pm.T @ yf[gl]
    o = np.exp((1.0 / pp) * np.log(dis_own[:, None] * agg + EPS_NUM))
    return o + (1 + eps) * x_own + mu


# ----------------------------------------------------------------------------
# public entry
# ----------------------------------------------------------------------------

_CACHE = {}


def _setup_jax():
    import jax
    cache = "/tmp/jax_neff_cache"
    os.makedirs(cache, exist_ok=True)
    try:
        jax.config.update("jax_compilation_cache_dir", cache)
        jax.config.update("jax_persistent_cache_min_entry_size_bytes", -1)
        jax.config.update("jax_persistent_cache_min_compile_time_secs", 0.0)
    except Exception:
        pass


def _pad_rows(a, rows, fill):
    if a.shape[0] == rows:
        return np.ascontiguousarray(a)
    out = np.full((rows,) + a.shape[1:], fill, a.dtype)
    out[: a.shape[0]] = a
    return out


def kernel(x, eps, p, edge_index):
    import jax
    _setup_jax()
    cfg = Cfg()
    x = np.asarray(x, np.float32)
    eps = np.asarray(eps, np.float32).reshape(1, 1)
    p = np.asarray(p, np.float32).reshape(1, 1)
    edge_index = np.asarray(edge_index)
    assert x.shape == (cfg.N, 128)

    offs, plans = plan_all(edge_index, cfg)
    x_sl = [
        _pad_rows(x[c * cfg.rpc_real:(c + 1) * cfg.rpc_real], cfg.rpc, 1e30)
        for c in range(cfg.ncores)
    ]
    # offsets padded so deg=0 beyond rpc_real
    off_sl = [_pad_rows(offs[c], cfg.rpc + 128, offs[c][-1])
              for c in range(cfg.ncores)]
    iota = _iota_tile()
    devices = jax.devices()[: cfg.ncores]

    # ---- L0: x min ----
    key0 = ("min", cfg.N)
    if key0 not in _CACHE:
        _CACHE[key0] = build_min(cfg)
    res0 = run_spmd(_CACHE[key0], [{"x_own": x_sl[c]}
                                   for c in range(cfg.ncores)])
    mu = np.array(min(float(r["xmin"][0, 0]) for r in res0),
                  np.float32).reshape(1, 1)

    # ---- L1: y ----
    key1 = ("y", cfg.N)
    if key1 not in _CACHE:
        _CACHE[key1] = build_y(cfg)
    res1 = run_spmd(_CACHE[key1], [
        {"x_own": x_sl[c], "off_own": off_sl[c], "mu": mu, "p": p}
        for c in range(cfg.ncores)
    ])
    y_full = np.concatenate(
        [res1[c]["y_own"][: cfg.rpc_real] for c in range(cfg.ncores)], axis=0)
    dis_sl = [res1[c]["dis_own"] for c in range(cfg.ncores)]

    # ---- L2 ----
    outs = [None] * cfg.ncores
    runners = []
    for c in range(cfg.ncores):
        key2 = ("p2", cfg.N, cfg.E, c,
                hash(plans[c].idx_wrapped.tobytes()),
                hash(plans[c].row_local.tobytes()))
        if key2 not in _CACHE:
            _CACHE[key2] = build_p2(cfg, plans[c])
        runners.append(SingleRunner(_CACHE[key2], devices[c]))
    for c in range(cfg.ncores):
        runners[c].put_inputs({
            "y_full": y_full, "gth_idx": plans[c].idx_wrapped,
            "row_local": plans[c].row_local, "iota": iota,
            "x_own": x_sl[c], "dis_own": dis_sl[c],
            "mu": mu, "p": p, "eps": eps,
        })
    futs = [runners[c].dispatch() for c in range(cfg.ncores)]
    jax.block_until_ready(futs)
    for c in range(cfg.ncores):
        outs[c] = runners[c].collect(futs[c])["out_own"][: cfg.rpc_real]
    return np.concatenate(outs, axis=0)


# revision 14
# speedup vs baseline: 2.7130x; 1.2127x over previous
"""GCNConv-variant Trainium2 kernel (8 NeuronCores, SPMD via bass/tile).

Math (from the reference):
    deg  = in-degree of col over all edges               [N]
    dis  = where(deg>0, deg^-1/2, 0)                     [N]
    pp   = sigmoid(p) + 1
    mu   = min(x)
    y    = dis * (x - mu + 1e-6)^pp                      [N,128]  (bf16)
    agg[i] = sum_{e: row[e]==i} y[col[e]]
    out  = (dis*agg + 1e-6)^(1/pp) + (1+eps)*x + mu

Distribution (3 launches, node ranges owned per core):
    L0 (uniform SPMD): per-core x-slice min -> host min -> mu.
    L1 (uniform SPMD): per-core deg (diff of host searchsorted offsets of
       its sorted owned-col list), dis, y for the owned range, y in bf16.
    L2 (one program per core): gather y rows by col (SWDGE dma_gather,
       single_packet=False, 4 queues, deep stage buffering); one-hot bf16
       matmuls (P^T @ y_batch) accumulate window segment-sums in PSUM;
       output transform reads PSUM directly (no SBUF accumulator) since
       edges are sorted group-major (grp, bank, row).
"""

import os
import sys
from contextlib import ExitStack

sys.path.insert(0, "/opt/trn_rl_repo")

import numpy as np
import ml_dtypes

import concourse.bass as bass
import concourse.bacc as bacc
import concourse.bass_isa as bass_isa
import concourse.mybir as mybir
import concourse.tile as tile

F32 = mybir.dt.float32
BF16 = mybir.dt.bfloat16
I16 = mybir.dt.int16
I32 = mybir.dt.int32
EPS_NUM = 1e-6
ALU = mybir.AluOpType
ACT = mybir.ActivationFunctionType


class Cfg:
    def __init__(self, N=100000, E=1600000, D=128, ncores=8, bank_rows=25000,
                 chunk=8, group=4, stage_bufs=10):
        assert D == 128
        self.N, self.E, self.D, self.ncores = N, E, D, ncores
        self.rpc_real = N // ncores            # owned rows per core
        assert self.rpc_real * ncores == N
        self.rpc = ((self.rpc_real + 127) // 128) * 128   # padded rows
        self.nwin = self.rpc // 128            # row windows per core
        self.bank_rows = bank_rows             # gather bank size (int16 limit)
        assert bank_rows <= 32768
        self.nbanks = (N + bank_rows - 1) // bank_rows
        self.chunk = chunk                     # gather batches per SWDGE call
        self.group = group                     # windows per PSUM bank tile
        self.ngrp = (self.nwin + group - 1) // group
        self.stage_bufs = stage_bufs


# ----------------------------------------------------------------------------
# host-side planning (pure index/layout work on edge_index; no float math)
# ----------------------------------------------------------------------------

def _wrap_idxs(idx_linear):
    """SWDGE index layout: slot i lives at [i%16, i//16], tiled to 128 parts."""
    n = len(idx_linear)
    assert n % 16 == 0
    a = np.zeros((16, n // 16), np.int16)
    ar = np.arange(n)
    a[ar % 16, ar // 16] = idx_linear.astype(np.int16)
    return np.tile(a, (8, 1))


class CorePlan:
    pass


def plan_core_p2(rows_local, cols, cfg: Cfg):
    """Plan one core's L2 schedule. rows_local in [0, rpc_real).

    Sort edges (bank, row); batches of <=128 edges within one (bank, win)
    so every batch targets exactly one window (no straddle) and PSUM
    accumulation groups within a bank tile open/close strictly
    sequentially. PSUM tiles keyed (bank, grp); flushed by DVE add into
    the SBUF agg accumulator; transform per grp follows the last flush
    that touches it.
    """
    G = cfg.group
    win = rows_local // 128
    grp = win // G
    bank = cols // cfg.bank_rows
    order = np.lexsort((rows_local, bank))
    r = rows_local[order]
    w = win[order]
    g = grp[order]
    b = bank[order]
    c = (cols - bank * cfg.bank_rows)[order]

    n = len(r)
    batches = []     # (grp, bank, win)
    rls = []
    idx_parts = []
    i = 0
    while i < n:
        j = min(i + 128, n)
        cut = j - i
        for k in range(i + 1, j):
            if w[k] != w[i] or b[k] != b[i]:
                cut = k - i
                break
        j = i + cut
        take = j - i
        rl = np.concatenate([r[i:j] - int(w[i]) * 128,
                             np.full(128 - take, -1, np.int64)])
        idx = np.concatenate([c[i:j], np.zeros(128 - take, np.int64)])
        batches.append((int(g[i]), int(b[i]), int(w[i])))
        rls.append(rl.astype(np.float32))
        idx_parts.append(idx)
        i = j

    nb = len(batches)
    plan = CorePlan()
    plan.nbatches = nb
    plan.batches = batches
    if nb == 0:
        plan.idx_wrapped = np.zeros((128, 8), np.int16)
        plan.row_local = np.zeros((128, 1), ml_dtypes.bfloat16)
        plan.chunks = []
        plan.first_touch = {}
        plan.last_touch = {}
        plan.flushes = {}
        plan.transforms = {}
        plan.grps_with_batches = set()
        plan.untouched_wins = list(range(cfg.nwin))
        return plan
    plan.idx_wrapped = _wrap_idxs(np.concatenate(idx_parts))
    plan.row_local = np.stack(rls, axis=1).astype(ml_dtypes.bfloat16)

    # gather chunks: runs of batches with same bank, up to cfg.chunk
    chunks = []   # (bank, bt0, nbatch)
    i = 0
    while i < nb:
        bb = batches[i][1]
        j = i
        while j < nb and batches[j][1] == bb and j - i < cfg.chunk:
            j += 1
        chunks.append((bb, i, j - i))
        i = j
    plan.chunks = chunks

    # per-(bank, win) first/last batch index -> matmul start/stop
    first_touch = {}
    last_touch = {}
    for bi, (gg, bb, ww) in enumerate(batches):
        key = (bb, ww)
        if key not in first_touch:
            first_touch[key] = bi
        last_touch[key] = bi
    plan.first_touch = first_touch
    plan.last_touch = last_touch

    # flushes: batch idx -> list of (bank, grp, [(win, is_first_flush)...])
    bg_last = {}
    bg_wins = {}
    win_first_bank = {}
    for bi, (gg, bb, ww) in enumerate(batches):
        bg_last[(bb, gg)] = bi
        bg_wins.setdefault((bb, gg), set()).add(ww)
        if ww not in win_first_bank:
            win_first_bank[ww] = bb
    flushes = {}
    for (bb, gg), last_bi in bg_last.items():
        wins = [(ww, win_first_bank[ww] == bb)
                for ww in sorted(bg_wins[(bb, gg)])]
        flushes.setdefault(last_bi, []).append((bb, gg, wins))
    plan.flushes = flushes
    plan.untouched_wins = [ww for ww in range(cfg.nwin)
                           if ww not in win_first_bank]

    # transforms: batch idx -> list of grps fully flushed after that batch
    grp_done = {}
    for (bb, gg), last_bi in bg_last.items():
        grp_done[gg] = max(grp_done.get(gg, -1), last_bi)
    transforms = {}
    for gg, bi in grp_done.items():
        transforms.setdefault(bi, []).append(gg)
    plan.transforms = transforms
    plan.grps_with_batches = set(grp_done.keys())
    return plan


def plan_all(edge_index, cfg: Cfg):
    """Shard edges. Returns (offsets per core, p2 plans per core)."""
    row = np.asarray(edge_index[0]).astype(np.int64)
    col = np.asarray(edge_index[1]).astype(np.int64)

    # ---- L1: per-core owned-col offsets (host indexing only) ----
    owner = col // cfg.rpc_real
    offs = []
    for cr in range(cfg.ncores):
        loc = np.sort(col[owner == cr] - cr * cfg.rpc_real)
        off = np.searchsorted(loc, np.arange(cfg.rpc + 1)).astype(np.float32)
        offs.append(off)

    # ---- L2: row shards ----
    rowner = row // cfg.rpc_real
    plans = []
    for cr in range(cfg.ncores):
        sel = rowner == cr
        plans.append(plan_core_p2((row[sel] - cr * cfg.rpc_real), col[sel], cfg))
    return offs, plans


# ----------------------------------------------------------------------------
# bass program builders
# ----------------------------------------------------------------------------

def _patch_act_tables(arch):
    """Steer the act-table chooser to the combined Ln+Exp set.

    The insert_act_table_loads pass picks the first act_info.json set
    containing each required function, which ping-pongs between the
    Ln-only and Exp-only sets (a ~1.3us table DMA per switch). Ln and Exp
    coexist in natural_log_exp_and_others; pruning them from the other
    sets (in the cached dict, preserving set ids) makes the chooser pick
    the combined set once. The emitted loads stay semantically correct —
    the chosen table genuinely contains every function used under it.
    """
    from concourse.hw_specs import get_activation_tables
    t = get_activation_tables(arch)
    combined = "natural_log_exp_and_others"
    if combined not in t:
        return
    for name, s in t.items():
        if name != combined:
            s.discard(ACT.Ln)
            s.discard(ACT.Exp)


def _mk_nc(**kw):
    nc = bacc.Bacc("TRN2", target_bir_lowering=False, debug=False,
                   enable_partition_id=False, **kw)
    _patch_act_tables(nc.m.arch)
    return nc


def _iota_tile():
    i = np.arange(128, dtype=np.float32)
    return np.tile(i, (128, 1)).astype(ml_dtypes.bfloat16)


def _dyn_loop(nc, tc, cpool, ctx, dynamic):
    if not dynamic:
        return
    rep_in = nc.dram_tensor("rep", [1, 1], I32, kind="ExternalInput")
    rep_sb = cpool.tile([1, 1], I32)
    nc.sync.dma_start(rep_sb[:], rep_in.ap()[:])
    regs = []
    for e in mybir.ALL_ENGINES:
        regs.append(nc.alloc_register(e, f"repreg_{e.name}"))
    nc.regs_load(bass.RegisterHandles(tuple(regs)), rep_sb[0:1, 0:1])
    rep_val = bass.make_scalar_value(
        bass.RegisterHandles(tuple(regs)), min_val=0, max_val=1 << 20)
    ctx.enter_context(tc.For_i(0, rep_val, 1))


def build_min(cfg: Cfg, rep=0):
    """Uniform SPMD program: x-slice min."""
    dynamic = rep == -1
    nc = _mk_nc()
    x = nc.dram_tensor("x_own", [cfg.rpc, 128], F32, kind="ExternalInput")
    xmin = nc.dram_tensor("xmin", [1, 1], F32, kind="ExternalOutput")
    NW = cfg.nwin
    with tile.TileContext(nc) as tc, ExitStack() as ctx:
        pool = ctx.enter_context(tc.tile_pool(name="m", bufs=3))
        cpool = ctx.enter_context(tc.tile_pool(name="mc", bufs=1))
        runmin = cpool.tile([128, 1], F32)
        _dyn_loop(nc, tc, cpool, ctx, dynamic)
        SW = 4
        nt4 = (NW // SW) * SW * 128
        xt4 = x.ap()[0:nt4, :].rearrange("(t w p) f -> t p w f", p=128, w=SW)
        xt3 = x.ap().rearrange("(w p) f -> w p f", p=128)
        nc.vector.memset(runmin[:], 1e30)
        for t in range(NW // SW):
            xt = pool.tile([128, SW, 128], F32)
            nc.sync.dma_start(xt[:], xt4[t])
            red = pool.tile([128, 1], F32)
            nc.vector.tensor_reduce(red[:], xt[:], mybir.AxisListType.XY, ALU.min)
            nc.vector.tensor_tensor(runmin[:], runmin[:], red[:], ALU.min)
        for w in range((NW // SW) * SW, NW):
            xt1 = pool.tile([128, 128], F32, name="xt1", tag="xt1")
            nc.sync.dma_start(xt1[:], xt3[w])
            red1 = pool.tile([128, 1], F32, name="red1", tag="red1")
            nc.vector.tensor_reduce(red1[:], xt1[:], mybir.AxisListType.X, ALU.min)
            nc.vector.tensor_tensor(runmin[:], runmin[:], red1[:], ALU.min)
        negmin = cpool.tile([128, 1], F32, name="negmin", tag="negmin")
        nc.vector.tensor_scalar(negmin[:], runmin[:], -1.0, None, ALU.mult)
        allmax = cpool.tile([128, 1], F32, name="allmax", tag="allmax")
        nc.gpsimd.partition_all_reduce(allmax[:], negmin[:], 128,
                                       bass_isa.ReduceOp.max)
        minv = cpool.tile([1, 1], F32, name="minv", tag="minv")
        nc.vector.tensor_scalar(minv[:], allmax[0:1, :], -1.0, None, ALU.mult)
        nc.sync.dma_start(xmin.ap()[:], minv[:])
    nc.compile()
    return nc


def build_y(cfg: Cfg, rep=0):
    """Uniform SPMD program: deg -> dis -> y (bf16) for the owned slice."""
    dynamic = rep == -1
    nc = _mk_nc()
    x = nc.dram_tensor("x_own", [cfg.rpc, 128], F32, kind="ExternalInput")
    off = nc.dram_tensor("off_own", [cfg.rpc + 128], F32, kind="ExternalInput")
    mu = nc.dram_tensor("mu", [1, 1], F32, kind="ExternalInput")
    p_in = nc.dram_tensor("p", [1, 1], F32, kind="ExternalInput")
    y = nc.dram_tensor("y_own", [cfg.rpc, 128], BF16, kind="ExternalOutput")
    dis_out = nc.dram_tensor("dis_own", [cfg.rpc], F32, kind="ExternalOutput")

    NW = cfg.nwin
    with tile.TileContext(nc) as tc, ExitStack() as ctx:
        pool = ctx.enter_context(tc.tile_pool(name="y", bufs=3))
        cpool = ctx.enter_context(tc.tile_pool(name="yc", bufs=1))

        # scalars
        psb = cpool.tile([1, 1], F32)
        nc.sync.dma_start(psb[:], p_in.ap()[:])
        sig = cpool.tile([1, 1], F32)
        nc.scalar.activation(sig[:], psb[:], ACT.Sigmoid)
        ppb = cpool.tile([128, 1], F32)
        nc.gpsimd.partition_broadcast(ppb[:], sig[:])
        pp_vec = cpool.tile([128, 1], F32)
        nc.vector.tensor_scalar(pp_vec[:], ppb[:], 1.0, None, ALU.add)
        musb = cpool.tile([1, 1], F32)
        nc.sync.dma_start(musb[:], mu.ap()[:])
        mub = cpool.tile([128, 1], F32)
        nc.gpsimd.partition_broadcast(mub[:], musb[:])
        cvec = cpool.tile([128, 1], F32)   # 1e-6 - mu
        nc.vector.tensor_scalar(cvec[:], mub[:], -1.0, EPS_NUM, ALU.mult, ALU.add)

        # ---- deg = off[n+1] - off[n]; ldis = -0.5*ln(max(deg,1)) - 100*(deg==0)
        offA = cpool.tile([128, NW], F32)
        nc.sync.dma_start(offA[:], off.ap()[1:cfg.rpc + 1]
                          .rearrange("(w p) -> p w", p=128))
        offB = cpool.tile([128, NW], F32)
        nc.sync.dma_start(offB[:], off.ap()[0:cfg.rpc]
                          .rearrange("(w p) -> p w", p=128))
        deg = cpool.tile([128, NW], F32)
        nc.vector.tensor_tensor(deg[:], offA[:], offB[:], ALU.subtract)
        mask = cpool.tile([128, NW], F32)
        nc.vector.tensor_scalar(mask[:], deg[:], 0.5, None, ALU.is_ge)
        degc = cpool.tile([128, NW], F32)
        nc.vector.tensor_scalar(degc[:], deg[:], 1.0, None, ALU.max)
        ldeg = cpool.tile([128, NW], F32)
        nc.scalar.activation(ldeg[:], degc[:], ACT.Ln)
        ldis = cpool.tile([128, NW], F32)
        nc.vector.tensor_scalar(ldis[:], ldeg[:], -0.5, None, ALU.mult)
        pen = cpool.tile([128, NW], F32)
        nc.vector.tensor_scalar(pen[:], mask[:], 100.0, -100.0, ALU.mult, ALU.add)
        nc.vector.tensor_tensor(ldis[:], ldis[:], pen[:], ALU.add)
        dis = cpool.tile([128, NW], F32)
        nc.scalar.activation(dis[:], ldis[:], ACT.Exp)
        nc.sync.dma_start(dis_out.ap().rearrange("(w p) -> p w", p=128), dis[:])

        _dyn_loop(nc, tc, cpool, ctx, dynamic)
        # y = exp(pp*ln(x - mu + eps) + ldis), 4-window supertiles
        SW = 4
        nt4 = (NW // SW) * SW * 128
        xt4 = x.ap()[0:nt4, :].rearrange("(t w p) f -> t p w f", p=128, w=SW)
        yt4 = y.ap()[0:nt4, :].rearrange("(t w p) f -> t p w f", p=128, w=SW)
        for t in range(NW // SW):
            xt = pool.tile([128, SW, 128], F32)
            nc.sync.dma_start(xt[:], xt4[t])
            t1 = pool.tile([128, SW, 128], F32)
            nc.scalar.activation(t1[:], xt[:], ACT.Ln, bias=cvec[:, 0:1])
            yt = pool.tile([128, SW, 128], BF16)
            for w in range(SW):
                nc.scalar.activation(yt[:, w, :], t1[:, w, :], ACT.Exp,
                                     bias=ldis[:, t * SW + w:t * SW + w + 1],
                                     scale=pp_vec[:, 0:1])
            nc.sync.dma_start(yt4[t], yt[:])
        xt3 = x.ap().rearrange("(w p) f -> w p f", p=128)
        yt3 = y.ap().rearrange("(w p) f -> w p f", p=128)
        for w in range((NW // SW) * SW, NW):
            xt1 = pool.tile([128, 128], F32, name="xt1", tag="xt1")
            nc.sync.dma_start(xt1[:], xt3[w])
            t1b = pool.tile([128, 128], F32, name="t1b", tag="t1b")
            nc.scalar.activation(t1b[:], xt1[:], ACT.Ln, bias=cvec[:, 0:1])
            ytb = pool.tile([128, 128], BF16, name="ytb", tag="ytb")
            nc.scalar.activation(ytb[:], t1b[:], ACT.Exp,
                                 bias=ldis[:, w:w + 1], scale=pp_vec[:, 0:1])
            nc.sync.dma_start(yt3[w], ytb[:])
    nc.compile()
    return nc


def build_p2(cfg: Cfg, plan: CorePlan, rep=0):
    """Per-core program: gather + one-hot matmul segment-sum + transform."""
    dynamic = rep == -1
    nc = _mk_nc(num_swdge_queues=4)
    NW, G = cfg.nwin, cfg.group
    y = nc.dram_tensor("y_full", [cfg.N, 128], BF16, kind="ExternalInput")
    nbat = max(plan.nbatches, 1)
    gidx = nc.dram_tensor("gth_idx", list(plan.idx_wrapped.shape), I16,
                          kind="ExternalInput")
    rloc = nc.dram_tensor("row_local", [128, nbat], BF16, kind="ExternalInput")
    iota_d = nc.dram_tensor("iota", [128, 128], BF16, kind="ExternalInput")
    x = nc.dram_tensor("x_own", [cfg.rpc, 128], F32, kind="ExternalInput")
    dis = nc.dram_tensor("dis_own", [cfg.rpc], F32, kind="ExternalInput")
    mu = nc.dram_tensor("mu", [1, 1], F32, kind="ExternalInput")
    p_in = nc.dram_tensor("p", [1, 1], F32, kind="ExternalInput")
    eps_in = nc.dram_tensor("eps", [1, 1], F32, kind="ExternalInput")
    out = nc.dram_tensor("out_own", [cfg.rpc, 128], F32, kind="ExternalOutput")

    with tile.TileContext(nc) as tc, ExitStack() as ctx:
        cpool = ctx.enter_context(tc.tile_pool(name="c", bufs=1))
        stg = ctx.enter_context(tc.tile_pool(name="stg", bufs=cfg.stage_bufs))
        ppool = ctx.enter_context(tc.tile_pool(name="ph", bufs=6))
        psum = ctx.enter_context(tc.tile_pool(name="ps", bufs=4, space="PSUM"))
        opool = ctx.enter_context(tc.tile_pool(name="op", bufs=4))

        # ---- constants / scalars ----
        idx_sb = cpool.tile(list(plan.idx_wrapped.shape), I16)
        nc.sync.dma_start(idx_sb[:], gidx.ap()[:])
        rl_sb = cpool.tile([128, nbat], BF16)
        nc.sync.dma_start(rl_sb[:], rloc.ap()[:])
        io_sb = cpool.tile([128, 128], BF16)
        nc.sync.dma_start(io_sb[:], iota_d.ap()[:])
        dis_sb = cpool.tile([128, NW], F32)
        nc.sync.dma_start(dis_sb[:], dis.ap().rearrange("(w p) -> p w", p=128))

        psb = cpool.tile([1, 1], F32)
        nc.sync.dma_start(psb[:], p_in.ap()[:])
        sig = cpool.tile([1, 1], F32)
        nc.scalar.activation(sig[:], psb[:], ACT.Sigmoid)
        pp1 = cpool.tile([1, 1], F32)
        nc.vector.tensor_scalar(pp1[:], sig[:], 1.0, None, ALU.add)
        ipps = cpool.tile([1, 1], F32)
        nc.vector.reciprocal(ipps[:], pp1[:])
        ipp_vec = cpool.tile([128, 1], F32)
        nc.gpsimd.partition_broadcast(ipp_vec[:], ipps[:])

        esb = cpool.tile([1, 1], F32)
        nc.sync.dma_start(esb[:], eps_in.ap()[:])
        eb = cpool.tile([128, 1], F32)
        nc.gpsimd.partition_broadcast(eb[:], esb[:])
        oneps = cpool.tile([128, 1], F32)
        nc.vector.tensor_scalar(oneps[:], eb[:], 1.0, None, ALU.add)
        musb = cpool.tile([1, 1], F32)
        nc.sync.dma_start(musb[:], mu.ap()[:])
        mu_vec = cpool.tile([128, 1], F32)
        nc.gpsimd.partition_broadcast(mu_vec[:], musb[:])
        epsv = cpool.tile([128, 1], F32)
        nc.vector.memset(epsv[:], EPS_NUM)

        loop_cm = ExitStack()
        if dynamic:
            rep_in = nc.dram_tensor("rep", [1, 1], I32, kind="ExternalInput")
            rep_sb = cpool.tile([1, 1], I32)
            nc.sync.dma_start(rep_sb[:], rep_in.ap()[:])
            regs = []
            for e in mybir.ALL_ENGINES:
                regs.append(nc.alloc_register(e, f"repreg_{e.name}"))
            nc.regs_load(bass.RegisterHandles(tuple(regs)), rep_sb[0:1, 0:1])
            rep_val = bass.make_scalar_value(
                bass.RegisterHandles(tuple(regs)), min_val=0, max_val=1 << 20)
            loop_cm.enter_context(tc.For_i(0, rep_val, 1))

        xt3 = x.ap().rearrange("(w p) f -> w p f", p=128)
        ot3 = out.ap().rearrange("(w p) f -> w p f", p=128)
        agg = cpool.tile([128, NW * 128], F32)
        # zero sources for overwrite-style TT flushes (TT never grabs the
        # DVE/GpSimd shared SBUF port, unlike memset/tensor_scalar, which
        # would starve SWDGE descriptor generation mid-gather)
        zcol = cpool.tile([128, 1], F32)
        nc.vector.memset(zcol[:], 0.0)
        ztile = cpool.tile([128, 128], F32)
        nc.vector.memset(ztile[:], 0.0)

        def transform_grp(gg):
            """Output transform for grp gg, reading the SBUF agg slice."""
            w0 = gg * G
            for w in range(w0, min(w0 + G, NW)):
                t1 = opool.tile([128, 128], F32, name="t1", tag="t1")
                nc.scalar.activation(t1[:], agg[:, w * 128:(w + 1) * 128],
                                     ACT.Ln, bias=epsv[:, 0:1],
                                     scale=dis_sb[:, w:w + 1])
                t2 = opool.tile([128, 128], F32, name="t2", tag="t2")
                nc.scalar.activation(t2[:], t1[:], ACT.Exp,
                                     scale=ipp_vec[:, 0:1])
                xt = opool.tile([128, 128], F32, name="xt", tag="xt")
                nc.sync.dma_start(xt[:], xt3[w])
                xw = opool.tile([128, 128], F32, name="xw", tag="xw")
                nc.scalar.activation(xw[:], xt[:], ACT.Identity,
                                     bias=mu_vec[:, 0:1], scale=oneps[:, 0:1])
                ot = opool.tile([128, 128], F32, name="ot", tag="ot")
                nc.vector.tensor_tensor(ot[:], t2[:], xw[:], ALU.add)
                nc.sync.dma_start(ot3[w], ot[:])

        with loop_cm:
            for ww in plan.untouched_wins:
                nc.vector.tensor_tensor(
                    agg[:, ww * 128:(ww + 1) * 128], ztile[:],
                    zcol[:, 0:1].broadcast_to([128, 128]), ALU.add)
            yap = y.ap()
            live_psum = {}
            for ci, (bank, bt0, cn) in enumerate(plan.chunks):
                stage = stg.tile([128, cfg.chunk, 128], BF16)
                s0 = bt0 * 128
                nc.gpsimd.dma_gather(
                    stage[:, 0:cn, :],
                    yap[bank * cfg.bank_rows:
                        min((bank + 1) * cfg.bank_rows, cfg.N), :],
                    idx_sb[:, s0 // 16: (s0 + cn * 128) // 16],
                    cn * 128, cn * 128, 128, elem_step=128,
                    queue_num=ci % 4, single_packet=False,
                )
                for k in range(cn):
                    bi = bt0 + k
                    gg, bb, w = plan.batches[bi]
                    key = (bb, gg)
                    if key not in live_psum:
                        live_psum[key] = psum.tile([128, G * 128], F32,
                                                   name="grp", tag="grp")
                    pt = live_psum[key]
                    P = ppool.tile([128, 128], BF16)
                    nc.vector.tensor_tensor(
                        P[:], io_sb[:],
                        rl_sb[:, bi:bi + 1].broadcast_to([128, 128]),
                        ALU.is_equal)
                    first = plan.first_touch[(bb, w)] == bi
                    last = plan.last_touch[(bb, w)] == bi
                    sl = pt[:, (w % G) * 128:(w % G) * 128 + 128]
                    nc.tensor.matmul(sl, P[:], stage[:, k, :],
                                     start=first, stop=last)
                    # flushes + transforms scheduled after this batch
                    for (fb, fg, wins) in plan.flushes.get(bi, []):
                        pt2 = live_psum.pop((fb, fg))
                        runs = []   # [start, end, is_first]
                        for ww, isf in wins:
                            if runs and ww == runs[-1][1] and \
                                    isf == runs[-1][2]:
                                runs[-1][1] = ww + 1
                            else:
                                runs.append([ww, ww + 1, isf])
                        for a, bnd, isf in runs:
                            fd = (bnd - a) * 128
                            psl = pt2[:, (a % G) * 128:(a % G) * 128 + fd]
                            if isf:
                                # first flush of these windows: agg = psum + 0
                                nc.vector.tensor_tensor(
                                    agg[:, a * 128:bnd * 128], psl,
                                    zcol[:, 0:1].broadcast_to([128, fd]),
                                    ALU.add)
                            else:
                                nc.vector.tensor_tensor(
                                    agg[:, a * 128:bnd * 128],
                                    agg[:, a * 128:bnd * 128], psl,
                                    ALU.add)
                    for gg2 in plan.transforms.get(bi, []):
                        transform_grp(gg2)
            # grps with no batches at all still need output
            for gg in range(cfg.ngrp):
                if gg not in plan.grps_with_batches:
                    transform_grp(gg)
    nc.compile()
    return nc


# ----------------------------------------------------------------------------
# PJRT runners
# ----------------------------------------------------------------------------

def _io_names(nc):
    in_names, out_names, out_avals = [], [], []
    import jax
    for alloc in nc.m.functions[0].allocations:
        if not isinstance(alloc, mybir.MemoryLocationSet):
            continue
        name = alloc.memorylocations[0].name
        if alloc.kind == "ExternalInput":
            if nc.partition_id_tensor is not None and \
                    name == nc.partition_id_tensor.name:
                continue
            in_names.append(name)
        elif alloc.kind == "ExternalOutput":
            out_names.append(name)
            out_avals.append(jax.core.ShapedArray(
                tuple(alloc.tensor_shape), mybir.dt.np(alloc.dtype)))
    return in_names, out_names, out_avals


def run_spmd(nc, in_maps):
    """Uniform program on len(in_maps) cores (the stock shard_map path)."""
    from concourse import bass2jax
    return bass2jax.run_bass_via_pjrt(nc, in_maps, n_cores=len(in_maps))


class SingleRunner:
    """One program pinned to one device; supports async dispatch."""

    def __init__(self, nc, device):
        import jax
        from concourse.bass2jax import _bass_exec_p, install_neuronx_cc_hook
        install_neuronx_cc_hook()
        assert nc.partition_id_tensor is None, \
            "per-core programs must not use partition id"
        self.nc, self.device = nc, device
        self.in_names, self.out_names, self.out_avals = _io_names(nc)
        all_in = tuple(self.in_names + self.out_names)
        out_avals = tuple(self.out_avals)
        out_names = tuple(self.out_names)

        def _body(*args):
            outs = _bass_exec_p.bind(
                *args, out_avals=out_avals, in_names=all_in,
                out_names=out_names, lowering_input_output_aliases=(),
                sim_require_finite=True, sim_require_nnan=True, nc=nc)
            return tuple(outs)

        n_params = len(self.in_names)
        donate = tuple(range(n_params, n_params + len(out_names)))
        self.fn = jax.jit(_body, donate_argnums=donate, keep_unused=True)
        self._dev_inputs = None

    def put_inputs(self, in_map):
        import jax
        self._dev_inputs = [jax.device_put(np.asarray(in_map[n]), self.device)
                            for n in self.in_names]
        jax.block_until_ready(self._dev_inputs)

    def dispatch(self):
        import jax
        import jax.numpy as jnp
        zeros = [jnp.zeros(a.shape, a.dtype, device=self.device)
                 for a in self.out_avals]
        return self.fn(*self._dev_inputs, *zeros)

    def collect(self, futs):
        return {n: np.asarray(f) for n, f in zip(self.out_names, futs)}


# ----------------------------------------------------------------------------
# numpy emulation of the planned L2 schedule (host-side logic check only)
# ----------------------------------------------------------------------------

def emulate_p2(cfg, plan, y_full, x_own, dis_own, mu, pp, eps):
    yf = np.asarray(y_full).astype(np.float32)
    agg = np.zeros((cfg.rpc, 128), np.float32)
    for bi, (gg, bb, w) in enumerate(plan.batches):
        s0 = bi * 128
        idx = np.zeros(128, np.int64)
        for i in range(128):
            idx[i] = plan.idx_wrapped[(s0 + i) % 16, (s0 + i) // 16]
        gl = bb * cfg.bank_rows + idx
        rl = plan.row_local[:, bi]
        Pm = (rl[:, None] == np.arange(128)[None, :]).astype(np.float32)
        agg[w * 128:(w + 1) * 128, :] += Pm.T @ yf[gl]
    o = np.exp((1.0 / pp) * np.log(dis_own[:, None] * agg + EPS_NUM))
    return o + (1 + eps) * x_own + mu


# ----------------------------------------------------------------------------
# public entry
# ----------------------------------------------------------------------------

_CACHE = {}


def _setup_jax():
    import jax
    cache = "/tmp/jax_neff_cache"
    os.makedirs(cache, exist_ok=True)
    try:
        jax.config.update("jax_compilation_cache_dir", cache)
        jax.config.update("jax_persistent_cache_min_entry_size_bytes", -1)
        jax.config.update("jax_persistent_cache_min_compile_time_secs", 0.0)
    except Exception:
        pass


def _pad_rows(a, rows, fill):
    if a.shape[0] == rows:
        return np.ascontiguousarray(a)
    out = np.full((rows,) + a.shape[1:], fill, a.dtype)
    out[: a.shape[0]] = a
    return out


def kernel(x, eps, p, edge_index):
    import jax
    _setup_jax()
    cfg = Cfg()
    x = np.asarray(x, np.float32)
    eps = np.asarray(eps, np.float32).reshape(1, 1)
    p = np.asarray(p, np.float32).reshape(1, 1)
    edge_index = np.asarray(edge_index)
    assert x.shape == (cfg.N, 128)

    offs, plans = plan_all(edge_index, cfg)
    x_sl = [
        _pad_rows(x[c * cfg.rpc_real:(c + 1) * cfg.rpc_real], cfg.rpc, 1e30)
        for c in range(cfg.ncores)
    ]
    # offsets padded so deg=0 beyond rpc_real
    off_sl = [_pad_rows(offs[c], cfg.rpc + 128, offs[c][-1])
              for c in range(cfg.ncores)]
    iota = _iota_tile()
    devices = jax.devices()[: cfg.ncores]

    # ---- L0: x min ----
    key0 = ("min", cfg.N)
    if key0 not in _CACHE:
        _CACHE[key0] = build_min(cfg)
    res0 = run_spmd(_CACHE[key0], [{"x_own": x_sl[c]}
                                   for c in range(cfg.ncores)])
    mu = np.array(min(float(r["xmin"][0, 0]) for r in res0),
                  np.float32).reshape(1, 1)

    # ---- L1: y ----
    key1 = ("y", cfg.N)
    if key1 not in _CACHE:
        _CACHE[key1] = build_y(cfg)
    res1 = run_spmd(_CACHE[key1], [
        {"x_own": x_sl[c], "off_own": off_sl[c], "mu": mu, "p": p}
        for c in range(cfg.ncores)
    ])
    y_full = np.concatenate(
        [res1[c]["y_own"][: cfg.rpc_real] for c in range(cfg.ncores)], axis=0)
    dis_sl = [res1[c]["dis_own"] for c in range(cfg.ncores)]

    # ---- L2 ----
    outs = [None] * cfg.ncores
    runners = []
    for c in range(cfg.ncores):
        key2 = ("p2", cfg.N, cfg.E, c,
                hash(plans[c].idx_wrapped.tobytes()),
                hash(plans[c].row_local.tobytes()))
        if key2 not in _CACHE:
            _CACHE[key2] = build_p2(cfg, plans[c])
        runners.append(SingleRunner(_CACHE[key2], devices[c]))
    for c in range(cfg.ncores):
        runners[c].put_inputs({
            "y_full": y_full, "gth_idx": plans[c].idx_wrapped,
            "row_local": plans[c].row_local, "iota": iota,
            "x_own": x_sl[c], "dis_own": dis_sl[c],
            "mu": mu, "p": p, "eps": eps,
        })
    futs = [runners[c].dispatch() for c in range(cfg.ncores)]
    jax.block_until_ready(futs)
    for c in range(cfg.ncores):
        outs[c] = runners[c].collect(futs[c])["out_own"][: cfg.rpc_real]
    return np.concatenate(outs, axis=0)


# revision 15
# speedup vs baseline: 3.1312x; 1.1542x over previous
"""GCNConv-variant Trainium2 kernel (8 NeuronCores, SPMD via bass/tile).

Math (from the reference):
    deg  = in-degree of col over all edges               [N]
    dis  = where(deg>0, deg^-1/2, 0)                     [N]
    pp   = sigmoid(p) + 1
    mu   = min(x)
    y    = dis * (x - mu + 1e-6)^pp                      [N,128]  (bf16)
    agg[i] = sum_{e: row[e]==i} y[col[e]]
    out  = (dis*agg + 1e-6)^(1/pp) + (1+eps)*x + mu

Distribution (3 launches, node ranges owned per core):
    L0 (uniform SPMD): per-core x-slice min -> host min -> mu.
    L1 (uniform SPMD): per-core deg (diff of host searchsorted offsets of
       its sorted owned-col list), dis, y for the owned range, y in bf16.
    L2 (one program per core): gather y rows by col (SWDGE dma_gather,
       single_packet=False, 4 queues, deep stage buffering); one-hot bf16
       matmuls (P^T @ y_batch) accumulate window segment-sums in PSUM;
       output transform reads PSUM directly (no SBUF accumulator) since
       edges are sorted group-major (grp, bank, row).
"""

import os
import sys
from contextlib import ExitStack

sys.path.insert(0, "/opt/trn_rl_repo")

import numpy as np
import ml_dtypes

import concourse.bass as bass
import concourse.bacc as bacc
import concourse.bass_isa as bass_isa
import concourse.mybir as mybir
import concourse.tile as tile

F32 = mybir.dt.float32
BF16 = mybir.dt.bfloat16
I16 = mybir.dt.int16
I32 = mybir.dt.int32
EPS_NUM = 1e-6
ALU = mybir.AluOpType
ACT = mybir.ActivationFunctionType


class Cfg:
    def __init__(self, N=100000, E=1600000, D=128, ncores=8, bank_rows=25000,
                 chunk=8, group=4, stage_bufs=10):
        assert D == 128
        self.N, self.E, self.D, self.ncores = N, E, D, ncores
        self.rpc_real = N // ncores            # owned rows per core
        assert self.rpc_real * ncores == N
        self.rpc = ((self.rpc_real + 127) // 128) * 128   # padded rows
        self.nwin = self.rpc // 128            # row windows per core
        self.bank_rows = bank_rows             # gather bank size (int16 limit)
        assert bank_rows <= 32768
        self.nbanks = (N + bank_rows - 1) // bank_rows
        self.chunk = chunk                     # gather batches per SWDGE call
        self.group = group                     # windows per PSUM bank tile
        self.ngrp = (self.nwin + group - 1) // group
        self.stage_bufs = stage_bufs


# ----------------------------------------------------------------------------
# host-side planning (pure index/layout work on edge_index; no float math)
# ----------------------------------------------------------------------------

def _wrap_idxs(idx_linear):
    """SWDGE index layout: slot i lives at [i%16, i//16], tiled to 128 parts."""
    n = len(idx_linear)
    assert n % 16 == 0
    a = np.zeros((16, n // 16), np.int16)
    ar = np.arange(n)
    a[ar % 16, ar // 16] = idx_linear.astype(np.int16)
    return np.tile(a, (8, 1))


class CorePlan:
    pass


def plan_core_p2(rows_local, cols, cfg: Cfg):
    """Plan one core's L2 schedule. rows_local in [0, rpc_real).

    Sort edges (bank, row); batches of <=128 edges within one (bank, win)
    so every batch targets exactly one window (no straddle) and PSUM
    accumulation groups within a bank tile open/close strictly
    sequentially. PSUM tiles keyed (bank, grp); flushed by DVE add into
    the SBUF agg accumulator; transform per grp follows the last flush
    that touches it.
    """
    G = cfg.group
    win = rows_local // 128
    grp = win // G
    bank = cols // cfg.bank_rows
    order = np.lexsort((rows_local, bank))
    r = rows_local[order]
    w = win[order]
    g = grp[order]
    b = bank[order]
    c = (cols - bank * cfg.bank_rows)[order]

    n = len(r)
    batches = []     # (grp, bank, win)
    rls = []
    idx_parts = []
    i = 0
    while i < n:
        j = min(i + 128, n)
        cut = j - i
        for k in range(i + 1, j):
            if w[k] != w[i] or b[k] != b[i]:
                cut = k - i
                break
        j = i + cut
        take = j - i
        rl = np.concatenate([r[i:j] - int(w[i]) * 128,
                             np.full(128 - take, -1, np.int64)])
        idx = np.concatenate([c[i:j], np.zeros(128 - take, np.int64)])
        batches.append((int(g[i]), int(b[i]), int(w[i])))
        rls.append(rl.astype(np.float32))
        idx_parts.append(idx)
        i = j

    nb = len(batches)
    plan = CorePlan()
    plan.nbatches = nb
    plan.batches = batches
    if nb == 0:
        plan.idx_wrapped = np.zeros((128, 8), np.int16)
        plan.row_local = np.zeros((128, 1), ml_dtypes.bfloat16)
        plan.chunks = []
        plan.first_touch = {}
        plan.last_touch = {}
        plan.flushes = {}
        plan.transforms = {}
        plan.grps_with_batches = set()
        plan.untouched_wins = list(range(cfg.nwin))
        return plan
    plan.idx_wrapped = _wrap_idxs(np.concatenate(idx_parts))
    plan.row_local = np.stack(rls, axis=1).astype(ml_dtypes.bfloat16)

    # gather chunks: runs of batches with same bank, up to cfg.chunk
    chunks = []   # (bank, bt0, nbatch)
    i = 0
    while i < nb:
        bb = batches[i][1]
        j = i
        while j < nb and batches[j][1] == bb and j - i < cfg.chunk:
            j += 1
        chunks.append((bb, i, j - i))
        i = j
    plan.chunks = chunks

    # per-(bank, win) first/last batch index -> matmul start/stop
    first_touch = {}
    last_touch = {}
    for bi, (gg, bb, ww) in enumerate(batches):
        key = (bb, ww)
        if key not in first_touch:
            first_touch[key] = bi
        last_touch[key] = bi
    plan.first_touch = first_touch
    plan.last_touch = last_touch

    # flushes: batch idx -> list of (bank, grp, [(win, is_first_flush)...])
    bg_last = {}
    bg_wins = {}
    win_first_bank = {}
    for bi, (gg, bb, ww) in enumerate(batches):
        bg_last[(bb, gg)] = bi
        bg_wins.setdefault((bb, gg), set()).add(ww)
        if ww not in win_first_bank:
            win_first_bank[ww] = bb
    flushes = {}
    for (bb, gg), last_bi in bg_last.items():
        wins = [(ww, win_first_bank[ww] == bb)
                for ww in sorted(bg_wins[(bb, gg)])]
        flushes.setdefault(last_bi, []).append((bb, gg, wins))
    plan.flushes = flushes
    plan.untouched_wins = [ww for ww in range(cfg.nwin)
                           if ww not in win_first_bank]

    # transforms: batch idx -> list of grps fully flushed after that batch
    grp_done = {}
    for (bb, gg), last_bi in bg_last.items():
        grp_done[gg] = max(grp_done.get(gg, -1), last_bi)
    transforms = {}
    for gg, bi in grp_done.items():
        transforms.setdefault(bi, []).append(gg)
    plan.transforms = transforms
    plan.grps_with_batches = set(grp_done.keys())
    return plan


def plan_all(edge_index, cfg: Cfg):
    """Shard edges. Returns (offsets per core, p2 plans per core)."""
    row = np.asarray(edge_index[0]).astype(np.int64)
    col = np.asarray(edge_index[1]).astype(np.int64)

    # ---- L1: per-core owned-col offsets (host indexing only) ----
    owner = col // cfg.rpc_real
    offs = []
    for cr in range(cfg.ncores):
        loc = np.sort(col[owner == cr] - cr * cfg.rpc_real)
        off = np.searchsorted(loc, np.arange(cfg.rpc + 1)).astype(np.float32)
        offs.append(off)

    # ---- L2: row shards ----
    rowner = row // cfg.rpc_real
    plans = []
    for cr in range(cfg.ncores):
        sel = rowner == cr
        plans.append(plan_core_p2((row[sel] - cr * cfg.rpc_real), col[sel], cfg))
    return offs, plans


# ----------------------------------------------------------------------------
# bass program builders
# ----------------------------------------------------------------------------

def _patch_act_tables(arch):
    """Steer the act-table chooser to the combined Ln+Exp set.

    The insert_act_table_loads pass picks the first act_info.json set
    containing each required function, which ping-pongs between the
    Ln-only and Exp-only sets (a ~1.3us table DMA per switch). Ln and Exp
    coexist in natural_log_exp_and_others; pruning them from the other
    sets (in the cached dict, preserving set ids) makes the chooser pick
    the combined set once. The emitted loads stay semantically correct —
    the chosen table genuinely contains every function used under it.
    """
    from concourse.hw_specs import get_activation_tables
    t = get_activation_tables(arch)
    combined = "natural_log_exp_and_others"
    if combined not in t:
        return
    for name, s in t.items():
        if name != combined:
            s.discard(ACT.Ln)
            s.discard(ACT.Exp)


def _mk_nc(**kw):
    nc = bacc.Bacc("TRN2", target_bir_lowering=False, debug=False,
                   enable_partition_id=False, **kw)
    _patch_act_tables(nc.m.arch)
    return nc


def _iota_tile():
    i = np.arange(128, dtype=np.float32)
    return np.tile(i, (128, 1)).astype(ml_dtypes.bfloat16)


def _dyn_loop(nc, tc, cpool, ctx, dynamic):
    if not dynamic:
        return
    rep_in = nc.dram_tensor("rep", [1, 1], I32, kind="ExternalInput")
    rep_sb = cpool.tile([1, 1], I32)
    nc.sync.dma_start(rep_sb[:], rep_in.ap()[:])
    regs = []
    for e in mybir.ALL_ENGINES:
        regs.append(nc.alloc_register(e, f"repreg_{e.name}"))
    nc.regs_load(bass.RegisterHandles(tuple(regs)), rep_sb[0:1, 0:1])
    rep_val = bass.make_scalar_value(
        bass.RegisterHandles(tuple(regs)), min_val=0, max_val=1 << 20)
    ctx.enter_context(tc.For_i(0, rep_val, 1))


def build_min(cfg: Cfg, rep=0):
    """Uniform SPMD program: x-slice min."""
    dynamic = rep == -1
    nc = _mk_nc()
    x = nc.dram_tensor("x_own", [cfg.rpc, 128], F32, kind="ExternalInput")
    xmin = nc.dram_tensor("xmin", [1, 1], F32, kind="ExternalOutput")
    NW = cfg.nwin
    with tile.TileContext(nc) as tc, ExitStack() as ctx:
        pool = ctx.enter_context(tc.tile_pool(name="m", bufs=3))
        cpool = ctx.enter_context(tc.tile_pool(name="mc", bufs=1))
        runmin = cpool.tile([128, 1], F32)
        _dyn_loop(nc, tc, cpool, ctx, dynamic)
        SW = 4
        nt4 = (NW // SW) * SW * 128
        xt4 = x.ap()[0:nt4, :].rearrange("(t w p) f -> t p w f", p=128, w=SW)
        xt3 = x.ap().rearrange("(w p) f -> w p f", p=128)
        nc.vector.memset(runmin[:], 1e30)
        for t in range(NW // SW):
            xt = pool.tile([128, SW, 128], F32)
            nc.sync.dma_start(xt[:], xt4[t])
            red = pool.tile([128, 1], F32)
            nc.vector.tensor_reduce(red[:], xt[:], mybir.AxisListType.XY, ALU.min)
            nc.vector.tensor_tensor(runmin[:], runmin[:], red[:], ALU.min)
        for w in range((NW // SW) * SW, NW):
            xt1 = pool.tile([128, 128], F32, name="xt1", tag="xt1")
            nc.sync.dma_start(xt1[:], xt3[w])
            red1 = pool.tile([128, 1], F32, name="red1", tag="red1")
            nc.vector.tensor_reduce(red1[:], xt1[:], mybir.AxisListType.X, ALU.min)
            nc.vector.tensor_tensor(runmin[:], runmin[:], red1[:], ALU.min)
        negmin = cpool.tile([128, 1], F32, name="negmin", tag="negmin")
        nc.vector.tensor_scalar(negmin[:], runmin[:], -1.0, None, ALU.mult)
        allmax = cpool.tile([128, 1], F32, name="allmax", tag="allmax")
        nc.gpsimd.partition_all_reduce(allmax[:], negmin[:], 128,
                                       bass_isa.ReduceOp.max)
        minv = cpool.tile([1, 1], F32, name="minv", tag="minv")
        nc.vector.tensor_scalar(minv[:], allmax[0:1, :], -1.0, None, ALU.mult)
        nc.sync.dma_start(xmin.ap()[:], minv[:])
    nc.compile()
    return nc


def build_y(cfg: Cfg, rep=0):
    """Uniform SPMD program: deg -> dis -> y (bf16) for the owned slice."""
    dynamic = rep == -1
    nc = _mk_nc()
    x = nc.dram_tensor("x_own", [cfg.rpc, 128], F32, kind="ExternalInput")
    off = nc.dram_tensor("off_own", [cfg.rpc + 128], F32, kind="ExternalInput")
    mu = nc.dram_tensor("mu", [1, 1], F32, kind="ExternalInput")
    p_in = nc.dram_tensor("p", [1, 1], F32, kind="ExternalInput")
    y = nc.dram_tensor("y_own", [cfg.rpc, 128], BF16, kind="ExternalOutput")
    dis_out = nc.dram_tensor("dis_own", [cfg.rpc], F32, kind="ExternalOutput")

    NW = cfg.nwin
    with tile.TileContext(nc) as tc, ExitStack() as ctx:
        pool = ctx.enter_context(tc.tile_pool(name="y", bufs=3))
        cpool = ctx.enter_context(tc.tile_pool(name="yc", bufs=1))

        # scalars
        psb = cpool.tile([1, 1], F32)
        nc.sync.dma_start(psb[:], p_in.ap()[:])
        sig = cpool.tile([1, 1], F32)
        nc.scalar.activation(sig[:], psb[:], ACT.Sigmoid)
        ppb = cpool.tile([128, 1], F32)
        nc.gpsimd.partition_broadcast(ppb[:], sig[:])
        pp_vec = cpool.tile([128, 1], F32)
        nc.vector.tensor_scalar(pp_vec[:], ppb[:], 1.0, None, ALU.add)
        musb = cpool.tile([1, 1], F32)
        nc.sync.dma_start(musb[:], mu.ap()[:])
        mub = cpool.tile([128, 1], F32)
        nc.gpsimd.partition_broadcast(mub[:], musb[:])
        cvec = cpool.tile([128, 1], F32)   # 1e-6 - mu
        nc.vector.tensor_scalar(cvec[:], mub[:], -1.0, EPS_NUM, ALU.mult, ALU.add)

        # ---- deg = off[n+1] - off[n]; ldis = -0.5*ln(max(deg,1)) - 100*(deg==0)
        offA = cpool.tile([128, NW], F32)
        nc.sync.dma_start(offA[:], off.ap()[1:cfg.rpc + 1]
                          .rearrange("(w p) -> p w", p=128))
        offB = cpool.tile([128, NW], F32)
        nc.sync.dma_start(offB[:], off.ap()[0:cfg.rpc]
                          .rearrange("(w p) -> p w", p=128))
        deg = cpool.tile([128, NW], F32)
        nc.vector.tensor_tensor(deg[:], offA[:], offB[:], ALU.subtract)
        mask = cpool.tile([128, NW], F32)
        nc.vector.tensor_scalar(mask[:], deg[:], 0.5, None, ALU.is_ge)
        degc = cpool.tile([128, NW], F32)
        nc.vector.tensor_scalar(degc[:], deg[:], 1.0, None, ALU.max)
        ldeg = cpool.tile([128, NW], F32)
        nc.scalar.activation(ldeg[:], degc[:], ACT.Ln)
        ldis = cpool.tile([128, NW], F32)
        nc.vector.tensor_scalar(ldis[:], ldeg[:], -0.5, None, ALU.mult)
        pen = cpool.tile([128, NW], F32)
        nc.vector.tensor_scalar(pen[:], mask[:], 100.0, -100.0, ALU.mult, ALU.add)
        nc.vector.tensor_tensor(ldis[:], ldis[:], pen[:], ALU.add)
        dis = cpool.tile([128, NW], F32)
        nc.scalar.activation(dis[:], ldis[:], ACT.Exp)
        nc.sync.dma_start(dis_out.ap().rearrange("(w p) -> p w", p=128), dis[:])

        _dyn_loop(nc, tc, cpool, ctx, dynamic)
        # y = exp(pp*ln(x - mu + eps) + ldis), 4-window supertiles
        SW = 4
        nt4 = (NW // SW) * SW * 128
        xt4 = x.ap()[0:nt4, :].rearrange("(t w p) f -> t p w f", p=128, w=SW)
        yt4 = y.ap()[0:nt4, :].rearrange("(t w p) f -> t p w f", p=128, w=SW)
        for t in range(NW // SW):
            xt = pool.tile([128, SW, 128], F32)
            nc.sync.dma_start(xt[:], xt4[t])
            t1 = pool.tile([128, SW, 128], F32)
            nc.scalar.activation(t1[:], xt[:], ACT.Ln, bias=cvec[:, 0:1])
            yt = pool.tile([128, SW, 128], BF16)
            for w in range(SW):
                nc.scalar.activation(yt[:, w, :], t1[:, w, :], ACT.Exp,
                                     bias=ldis[:, t * SW + w:t * SW + w + 1],
                                     scale=pp_vec[:, 0:1])
            nc.sync.dma_start(yt4[t], yt[:])
        xt3 = x.ap().rearrange("(w p) f -> w p f", p=128)
        yt3 = y.ap().rearrange("(w p) f -> w p f", p=128)
        for w in range((NW // SW) * SW, NW):
            xt1 = pool.tile([128, 128], F32, name="xt1", tag="xt1")
            nc.sync.dma_start(xt1[:], xt3[w])
            t1b = pool.tile([128, 128], F32, name="t1b", tag="t1b")
            nc.scalar.activation(t1b[:], xt1[:], ACT.Ln, bias=cvec[:, 0:1])
            ytb = pool.tile([128, 128], BF16, name="ytb", tag="ytb")
            nc.scalar.activation(ytb[:], t1b[:], ACT.Exp,
                                 bias=ldis[:, w:w + 1], scale=pp_vec[:, 0:1])
            nc.sync.dma_start(yt3[w], ytb[:])
    nc.compile()
    return nc


def build_p2(cfg: Cfg, plan: CorePlan, rep=0):
    """Per-core program: gather + one-hot matmul segment-sum + transform."""
    dynamic = rep == -1
    nc = _mk_nc(num_swdge_queues=4)
    NW, G = cfg.nwin, cfg.group
    y = nc.dram_tensor("y_full", [cfg.N, 128], BF16, kind="ExternalInput")
    nbat = max(plan.nbatches, 1)
    gidx = nc.dram_tensor("gth_idx", list(plan.idx_wrapped.shape), I16,
                          kind="ExternalInput")
    rloc = nc.dram_tensor("row_local", [128, nbat], BF16, kind="ExternalInput")
    iota_d = nc.dram_tensor("iota", [128, 128], BF16, kind="ExternalInput")
    x = nc.dram_tensor("x_own", [cfg.rpc, 128], F32, kind="ExternalInput")
    dis = nc.dram_tensor("dis_own", [cfg.rpc], F32, kind="ExternalInput")
    mu = nc.dram_tensor("mu", [1, 1], F32, kind="ExternalInput")
    p_in = nc.dram_tensor("p", [1, 1], F32, kind="ExternalInput")
    eps_in = nc.dram_tensor("eps", [1, 1], F32, kind="ExternalInput")
    out = nc.dram_tensor("out_own", [cfg.rpc, 128], F32, kind="ExternalOutput")

    with tile.TileContext(nc) as tc, ExitStack() as ctx:
        cpool = ctx.enter_context(tc.tile_pool(name="c", bufs=1))
        stg = ctx.enter_context(tc.tile_pool(name="stg", bufs=cfg.stage_bufs))
        ppool = ctx.enter_context(tc.tile_pool(name="ph", bufs=6))
        psum = ctx.enter_context(tc.tile_pool(name="ps", bufs=4, space="PSUM"))
        opool = ctx.enter_context(tc.tile_pool(name="op", bufs=4))

        # ---- constants / scalars ----
        idx_sb = cpool.tile(list(plan.idx_wrapped.shape), I16)
        nc.sync.dma_start(idx_sb[:], gidx.ap()[:])
        rl_sb = cpool.tile([128, nbat], BF16)
        nc.sync.dma_start(rl_sb[:], rloc.ap()[:])
        io_sb = cpool.tile([128, 128], BF16)
        nc.sync.dma_start(io_sb[:], iota_d.ap()[:])
        dis_sb = cpool.tile([128, NW], F32)
        nc.sync.dma_start(dis_sb[:], dis.ap().rearrange("(w p) -> p w", p=128))

        psb = cpool.tile([1, 1], F32)
        nc.sync.dma_start(psb[:], p_in.ap()[:])
        sig = cpool.tile([1, 1], F32)
        nc.scalar.activation(sig[:], psb[:], ACT.Sigmoid)
        pp1 = cpool.tile([1, 1], F32)
        nc.vector.tensor_scalar(pp1[:], sig[:], 1.0, None, ALU.add)
        ipps = cpool.tile([1, 1], F32)
        nc.vector.reciprocal(ipps[:], pp1[:])
        ipp_vec = cpool.tile([128, 1], F32)
        nc.gpsimd.partition_broadcast(ipp_vec[:], ipps[:])

        esb = cpool.tile([1, 1], F32)
        nc.sync.dma_start(esb[:], eps_in.ap()[:])
        eb = cpool.tile([128, 1], F32)
        nc.gpsimd.partition_broadcast(eb[:], esb[:])
        oneps = cpool.tile([128, 1], F32)
        nc.vector.tensor_scalar(oneps[:], eb[:], 1.0, None, ALU.add)
        musb = cpool.tile([1, 1], F32)
        nc.sync.dma_start(musb[:], mu.ap()[:])
        mu_vec = cpool.tile([128, 1], F32)
        nc.gpsimd.partition_broadcast(mu_vec[:], musb[:])
        epsv = cpool.tile([128, 1], F32)
        nc.vector.memset(epsv[:], EPS_NUM)

        loop_cm = ExitStack()
        if dynamic:
            rep_in = nc.dram_tensor("rep", [1, 1], I32, kind="ExternalInput")
            rep_sb = cpool.tile([1, 1], I32)
            nc.sync.dma_start(rep_sb[:], rep_in.ap()[:])
            regs = []
            for e in mybir.ALL_ENGINES:
                regs.append(nc.alloc_register(e, f"repreg_{e.name}"))
            nc.regs_load(bass.RegisterHandles(tuple(regs)), rep_sb[0:1, 0:1])
            rep_val = bass.make_scalar_value(
                bass.RegisterHandles(tuple(regs)), min_val=0, max_val=1 << 20)
            loop_cm.enter_context(tc.For_i(0, rep_val, 1))

        xt3 = x.ap().rearrange("(w p) f -> w p f", p=128)
        agg = cpool.tile([128, NW * 128], F32)
        obuf = cpool.tile([128, NW * 128], F32, name="obuf", tag="obuf")
        # zero sources for overwrite-style TT flushes (TT never grabs the
        # DVE/GpSimd shared SBUF port, unlike memset/tensor_scalar, which
        # would starve SWDGE descriptor generation mid-gather)
        zcol = cpool.tile([128, 1], F32)
        nc.vector.memset(zcol[:], 0.0)
        ztile = cpool.tile([128, 128], F32)
        nc.vector.memset(ztile[:], 0.0)

        def transform_grp(gg):
            """Output transform for grp gg: agg slice -> obuf slice.

            All HBM output traffic is deferred to one final DMA — interleaved
            64KB writes between the random gather reads thrash the HBM
            read/write turnaround when two cores share a stack.
            """
            w0 = gg * G
            wn = min(w0 + G, NW) - w0
            xt = opool.tile([128, G, 128], F32, name="xt", tag="xt")
            nc.sync.dma_start(xt[:, 0:wn, :],
                              x.ap()[w0 * 128:(w0 + wn) * 128, :]
                              .rearrange("(w p) f -> p w f", p=128))
            for w in range(w0, w0 + wn):
                t1 = opool.tile([128, 128], F32, name="t1", tag="t1")
                nc.scalar.activation(t1[:], agg[:, w * 128:(w + 1) * 128],
                                     ACT.Ln, bias=epsv[:, 0:1],
                                     scale=dis_sb[:, w:w + 1])
                t2 = opool.tile([128, 128], F32, name="t2", tag="t2")
                nc.scalar.activation(t2[:], t1[:], ACT.Exp,
                                     scale=ipp_vec[:, 0:1])
                xw = opool.tile([128, 128], F32, name="xw", tag="xw")
                nc.scalar.activation(xw[:], xt[:, w - w0, :], ACT.Identity,
                                     bias=mu_vec[:, 0:1], scale=oneps[:, 0:1])
                nc.vector.tensor_tensor(obuf[:, w * 128:(w + 1) * 128],
                                        t2[:], xw[:], ALU.add)

        with loop_cm:
            for ww in plan.untouched_wins:
                nc.vector.tensor_tensor(
                    agg[:, ww * 128:(ww + 1) * 128], ztile[:],
                    zcol[:, 0:1].broadcast_to([128, 128]), ALU.add)
            yap = y.ap()
            live_psum = {}
            for ci, (bank, bt0, cn) in enumerate(plan.chunks):
                stage = stg.tile([128, cfg.chunk, 128], BF16)
                s0 = bt0 * 128
                nc.gpsimd.dma_gather(
                    stage[:, 0:cn, :],
                    yap[bank * cfg.bank_rows:
                        min((bank + 1) * cfg.bank_rows, cfg.N), :],
                    idx_sb[:, s0 // 16: (s0 + cn * 128) // 16],
                    cn * 128, cn * 128, 128, elem_step=128,
                    queue_num=ci % 4, single_packet=False,
                )
                for k in range(cn):
                    bi = bt0 + k
                    gg, bb, w = plan.batches[bi]
                    key = (bb, gg)
                    if key not in live_psum:
                        live_psum[key] = psum.tile([128, G * 128], F32,
                                                   name="grp", tag="grp")
                    pt = live_psum[key]
                    P = ppool.tile([128, 128], BF16)
                    nc.vector.tensor_tensor(
                        P[:], io_sb[:],
                        rl_sb[:, bi:bi + 1].broadcast_to([128, 128]),
                        ALU.is_equal)
                    first = plan.first_touch[(bb, w)] == bi
                    last = plan.last_touch[(bb, w)] == bi
                    sl = pt[:, (w % G) * 128:(w % G) * 128 + 128]
                    nc.tensor.matmul(sl, P[:], stage[:, k, :],
                                     start=first, stop=last)
                    # flushes + transforms scheduled after this batch
                    for (fb, fg, wins) in plan.flushes.get(bi, []):
                        pt2 = live_psum.pop((fb, fg))
                        runs = []   # [start, end, is_first]
                        for ww, isf in wins:
                            if runs and ww == runs[-1][1] and \
                                    isf == runs[-1][2]:
                                runs[-1][1] = ww + 1
                            else:
                                runs.append([ww, ww + 1, isf])
                        for a, bnd, isf in runs:
                            fd = (bnd - a) * 128
                            psl = pt2[:, (a % G) * 128:(a % G) * 128 + fd]
                            if isf:
                                # first flush of these windows: agg = psum + 0
                                nc.vector.tensor_tensor(
                                    agg[:, a * 128:bnd * 128], psl,
                                    zcol[:, 0:1].broadcast_to([128, fd]),
                                    ALU.add)
                            else:
                                nc.vector.tensor_tensor(
                                    agg[:, a * 128:bnd * 128],
                                    agg[:, a * 128:bnd * 128], psl,
                                    ALU.add)
                    for gg2 in plan.transforms.get(bi, []):
                        transform_grp(gg2)
            # grps with no batches at all still need output
            for gg in range(cfg.ngrp):
                if gg not in plan.grps_with_batches:
                    transform_grp(gg)
            nc.sync.dma_start(
                out.ap().rearrange("(w p) f -> p w f", p=128),
                obuf[:].rearrange("p (w f) -> p w f", f=128))
    nc.compile()
    return nc


# ----------------------------------------------------------------------------
# PJRT runners
# ----------------------------------------------------------------------------

def _io_names(nc):
    in_names, out_names, out_avals = [], [], []
    import jax
    for alloc in nc.m.functions[0].allocations:
        if not isinstance(alloc, mybir.MemoryLocationSet):
            continue
        name = alloc.memorylocations[0].name
        if alloc.kind == "ExternalInput":
            if nc.partition_id_tensor is not None and \
                    name == nc.partition_id_tensor.name:
                continue
            in_names.append(name)
        elif alloc.kind == "ExternalOutput":
            out_names.append(name)
            out_avals.append(jax.core.ShapedArray(
                tuple(alloc.tensor_shape), mybir.dt.np(alloc.dtype)))
    return in_names, out_names, out_avals


def run_spmd(nc, in_maps):
    """Uniform program on len(in_maps) cores (the stock shard_map path)."""
    from concourse import bass2jax
    return bass2jax.run_bass_via_pjrt(nc, in_maps, n_cores=len(in_maps))


class SingleRunner:
    """One program pinned to one device; supports async dispatch."""

    def __init__(self, nc, device):
        import jax
        from concourse.bass2jax import _bass_exec_p, install_neuronx_cc_hook
        install_neuronx_cc_hook()
        assert nc.partition_id_tensor is None, \
            "per-core programs must not use partition id"
        self.nc, self.device = nc, device
        self.in_names, self.out_names, self.out_avals = _io_names(nc)
        all_in = tuple(self.in_names + self.out_names)
        out_avals = tuple(self.out_avals)
        out_names = tuple(self.out_names)

        def _body(*args):
            outs = _bass_exec_p.bind(
                *args, out_avals=out_avals, in_names=all_in,
                out_names=out_names, lowering_input_output_aliases=(),
                sim_require_finite=True, sim_require_nnan=True, nc=nc)
            return tuple(outs)

        n_params = len(self.in_names)
        donate = tuple(range(n_params, n_params + len(out_names)))
        self.fn = jax.jit(_body, donate_argnums=donate, keep_unused=True)
        self._dev_inputs = None

    def put_inputs(self, in_map):
        import jax
        self._dev_inputs = [jax.device_put(np.asarray(in_map[n]), self.device)
                            for n in self.in_names]
        jax.block_until_ready(self._dev_inputs)

    def dispatch(self):
        import jax
        import jax.numpy as jnp
        zeros = [jnp.zeros(a.shape, a.dtype, device=self.device)
                 for a in self.out_avals]
        return self.fn(*self._dev_inputs, *zeros)

    def collect(self, futs):
        return {n: np.asarray(f) for n, f in zip(self.out_names, futs)}


# ----------------------------------------------------------------------------
# numpy emulation of the planned L2 schedule (host-side logic check only)
# ----------------------------------------------------------------------------

def emulate_p2(cfg, plan, y_full, x_own, dis_own, mu, pp, eps):
    yf = np.asarray(y_full).astype(np.float32)
    agg = np.zeros((cfg.rpc, 128), np.float32)
    for bi, (gg, bb, w) in enumerate(plan.batches):
        s0 = bi * 128
        idx = np.zeros(128, np.int64)
        for i in range(128):
            idx[i] = plan.idx_wrapped[(s0 + i) % 16, (s0 + i) // 16]
        gl = bb * cfg.bank_rows + idx
        rl = plan.row_local[:, bi]
        Pm = (rl[:, None] == np.arange(128)[None, :]).astype(np.float32)
        agg[w * 128:(w + 1) * 128, :] += Pm.T @ yf[gl]
    o = np.exp((1.0 / pp) * np.log(dis_own[:, None] * agg + EPS_NUM))
    return o + (1 + eps) * x_own + mu


# ----------------------------------------------------------------------------
# public entry
# ----------------------------------------------------------------------------

_CACHE = {}


def _setup_jax():
    import jax
    cache = "/tmp/jax_neff_cache"
    os.makedirs(cache, exist_ok=True)
    try:
        jax.config.update("jax_compilation_cache_dir", cache)
        jax.config.update("jax_persistent_cache_min_entry_size_bytes", -1)
        jax.config.update("jax_persistent_cache_min_compile_time_secs", 0.0)
    except Exception:
        pass


def _pad_rows(a, rows, fill):
    if a.shape[0] == rows:
        return np.ascontiguousarray(a)
    out = np.full((rows,) + a.shape[1:], fill, a.dtype)
    out[: a.shape[0]] = a
    return out


def kernel(x, eps, p, edge_index):
    import jax
    _setup_jax()
    cfg = Cfg()
    x = np.asarray(x, np.float32)
    eps = np.asarray(eps, np.float32).reshape(1, 1)
    p = np.asarray(p, np.float32).reshape(1, 1)
    edge_index = np.asarray(edge_index)
    assert x.shape == (cfg.N, 128)

    offs, plans = plan_all(edge_index, cfg)
    x_sl = [
        _pad_rows(x[c * cfg.rpc_real:(c + 1) * cfg.rpc_real], cfg.rpc, 1e30)
        for c in range(cfg.ncores)
    ]
    # offsets padded so deg=0 beyond rpc_real
    off_sl = [_pad_rows(offs[c], cfg.rpc + 128, offs[c][-1])
              for c in range(cfg.ncores)]
    iota = _iota_tile()
    devices = jax.devices()[: cfg.ncores]

    # ---- L0: x min ----
    key0 = ("min", cfg.N)
    if key0 not in _CACHE:
        _CACHE[key0] = build_min(cfg)
    res0 = run_spmd(_CACHE[key0], [{"x_own": x_sl[c]}
                                   for c in range(cfg.ncores)])
    mu = np.array(min(float(r["xmin"][0, 0]) for r in res0),
                  np.float32).reshape(1, 1)

    # ---- L1: y ----
    key1 = ("y", cfg.N)
    if key1 not in _CACHE:
        _CACHE[key1] = build_y(cfg)
    res1 = run_spmd(_CACHE[key1], [
        {"x_own": x_sl[c], "off_own": off_sl[c], "mu": mu, "p": p}
        for c in range(cfg.ncores)
    ])
    y_full = np.concatenate(
        [res1[c]["y_own"][: cfg.rpc_real] for c in range(cfg.ncores)], axis=0)
    dis_sl = [res1[c]["dis_own"] for c in range(cfg.ncores)]

    # ---- L2 ----
    outs = [None] * cfg.ncores
    runners = []
    for c in range(cfg.ncores):
        key2 = ("p2", cfg.N, cfg.E, c,
                hash(plans[c].idx_wrapped.tobytes()),
                hash(plans[c].row_local.tobytes()))
        if key2 not in _CACHE:
            _CACHE[key2] = build_p2(cfg, plans[c])
        runners.append(SingleRunner(_CACHE[key2], devices[c]))
    for c in range(cfg.ncores):
        runners[c].put_inputs({
            "y_full": y_full, "gth_idx": plans[c].idx_wrapped,
            "row_local": plans[c].row_local, "iota": iota,
            "x_own": x_sl[c], "dis_own": dis_sl[c],
            "mu": mu, "p": p, "eps": eps,
        })
    futs = [runners[c].dispatch() for c in range(cfg.ncores)]
    jax.block_until_ready(futs)
    for c in range(cfg.ncores):
        outs[c] = runners[c].collect(futs[c])["out_own"][: cfg.rpc_real]
    return np.concatenate(outs, axis=0)


# revision 16
# speedup vs baseline: 3.5575x; 1.1361x over previous
"""GCNConv-variant Trainium2 kernel (8 NeuronCores, SPMD via bass/tile).

Math (from the reference):
    deg  = in-degree of col over all edges               [N]
    dis  = where(deg>0, deg^-1/2, 0)                     [N]
    pp   = sigmoid(p) + 1
    mu   = min(x)
    y    = dis * (x - mu + 1e-6)^pp                      [N,128]  (bf16)
    agg[i] = sum_{e: row[e]==i} y[col[e]]
    out  = (dis*agg + 1e-6)^(1/pp) + (1+eps)*x + mu

Distribution (3 launches, node ranges owned per core):
    L0 (uniform SPMD): per-core x-slice min -> host min -> mu.
    L1 (uniform SPMD): per-core deg (diff of host searchsorted offsets of
       its sorted owned-col list), dis, y for the owned range, y in bf16.
    L2 (one program per core): gather y rows by col (SWDGE dma_gather,
       single_packet=False, 4 queues, deep stage buffering); one-hot bf16
       matmuls (P^T @ y_batch) accumulate window segment-sums in PSUM;
       output transform reads PSUM directly (no SBUF accumulator) since
       edges are sorted group-major (grp, bank, row).
"""

import os
import sys
from contextlib import ExitStack

sys.path.insert(0, "/opt/trn_rl_repo")

import numpy as np
import ml_dtypes

import concourse.bass as bass
import concourse.bacc as bacc
import concourse.bass_isa as bass_isa
import concourse.mybir as mybir
import concourse.tile as tile

F32 = mybir.dt.float32
BF16 = mybir.dt.bfloat16
I16 = mybir.dt.int16
I32 = mybir.dt.int32
EPS_NUM = 1e-6
ALU = mybir.AluOpType
ACT = mybir.ActivationFunctionType


class Cfg:
    def __init__(self, N=100000, E=1600000, D=128, ncores=8, bank_rows=25000,
                 chunk=8, group=4, stage_bufs=10):
        assert D == 128
        self.N, self.E, self.D, self.ncores = N, E, D, ncores
        self.rpc_real = N // ncores            # owned rows per core
        assert self.rpc_real * ncores == N
        self.rpc = ((self.rpc_real + 127) // 128) * 128   # padded rows
        self.nwin = self.rpc // 128            # row windows per core
        self.bank_rows = bank_rows             # gather bank size (int16 limit)
        assert bank_rows <= 32768
        self.nbanks = (N + bank_rows - 1) // bank_rows
        self.chunk = chunk                     # gather batches per SWDGE call
        self.group = group                     # windows per PSUM bank tile
        self.ngrp = (self.nwin + group - 1) // group
        self.stage_bufs = stage_bufs


# ----------------------------------------------------------------------------
# host-side planning (pure index/layout work on edge_index; no float math)
# ----------------------------------------------------------------------------

def _wrap_idxs(idx_linear):
    """SWDGE index layout: slot i lives at [i%16, i//16], tiled to 128 parts."""
    n = len(idx_linear)
    assert n % 16 == 0
    a = np.zeros((16, n // 16), np.int16)
    ar = np.arange(n)
    a[ar % 16, ar // 16] = idx_linear.astype(np.int16)
    return np.tile(a, (8, 1))


class CorePlan:
    pass


def plan_core_p2(rows_local, cols, cfg: Cfg):
    """Plan one core's L2 schedule. rows_local in [0, rpc_real).

    Sort edges (bank, row); batches of <=128 edges within one (bank, win)
    so every batch targets exactly one window (no straddle) and PSUM
    accumulation groups within a bank tile open/close strictly
    sequentially. PSUM tiles keyed (bank, grp); flushed by DVE add into
    the SBUF agg accumulator; transform per grp follows the last flush
    that touches it.
    """
    G = cfg.group
    win = rows_local // 128
    grp = win // G
    bank = cols // cfg.bank_rows
    order = np.lexsort((rows_local, bank))
    r = rows_local[order]
    w = win[order]
    g = grp[order]
    b = bank[order]
    c = (cols - bank * cfg.bank_rows)[order]

    n = len(r)
    batches = []     # (grp, bank, win)
    rls = []
    idx_parts = []
    i = 0
    while i < n:
        j = min(i + 128, n)
        cut = j - i
        for k in range(i + 1, j):
            if w[k] != w[i] or b[k] != b[i]:
                cut = k - i
                break
        j = i + cut
        take = j - i
        rl = np.concatenate([r[i:j] - int(w[i]) * 128,
                             np.full(128 - take, -1, np.int64)])
        idx = np.concatenate([c[i:j], np.zeros(128 - take, np.int64)])
        batches.append((int(g[i]), int(b[i]), int(w[i])))
        rls.append(rl.astype(np.float32))
        idx_parts.append(idx)
        i = j

    nb = len(batches)
    plan = CorePlan()
    plan.nbatches = nb
    plan.batches = batches
    if nb == 0:
        plan.idx_wrapped = np.zeros((128, 8), np.int16)
        plan.row_local = np.zeros((128, 1), ml_dtypes.bfloat16)
        plan.chunks = []
        plan.first_touch = {}
        plan.last_touch = {}
        plan.flushes = {}
        plan.transforms = {}
        plan.grps_with_batches = set()
        plan.untouched_wins = list(range(cfg.nwin))
        return plan
    plan.idx_wrapped = _wrap_idxs(np.concatenate(idx_parts))
    plan.row_local = np.stack(rls, axis=1).astype(ml_dtypes.bfloat16)

    # gather chunks: runs of batches with same bank, up to cfg.chunk
    chunks = []   # (bank, bt0, nbatch)
    i = 0
    while i < nb:
        bb = batches[i][1]
        j = i
        while j < nb and batches[j][1] == bb and j - i < cfg.chunk:
            j += 1
        chunks.append((bb, i, j - i))
        i = j
    plan.chunks = chunks

    # per-(bank, win) first/last batch index -> matmul start/stop
    first_touch = {}
    last_touch = {}
    for bi, (gg, bb, ww) in enumerate(batches):
        key = (bb, ww)
        if key not in first_touch:
            first_touch[key] = bi
        last_touch[key] = bi
    plan.first_touch = first_touch
    plan.last_touch = last_touch

    # flushes: batch idx -> list of (bank, grp, [(win, is_first_flush)...])
    bg_last = {}
    bg_wins = {}
    win_first_bank = {}
    for bi, (gg, bb, ww) in enumerate(batches):
        bg_last[(bb, gg)] = bi
        bg_wins.setdefault((bb, gg), set()).add(ww)
        if ww not in win_first_bank:
            win_first_bank[ww] = bb
    flushes = {}
    for (bb, gg), last_bi in bg_last.items():
        wins = [(ww, win_first_bank[ww] == bb)
                for ww in sorted(bg_wins[(bb, gg)])]
        flushes.setdefault(last_bi, []).append((bb, gg, wins))
    plan.flushes = flushes
    plan.untouched_wins = [ww for ww in range(cfg.nwin)
                           if ww not in win_first_bank]

    # transforms: batch idx -> list of grps fully flushed after that batch
    grp_done = {}
    for (bb, gg), last_bi in bg_last.items():
        grp_done[gg] = max(grp_done.get(gg, -1), last_bi)
    transforms = {}
    for gg, bi in grp_done.items():
        transforms.setdefault(bi, []).append(gg)
    plan.transforms = transforms
    plan.grps_with_batches = set(grp_done.keys())
    return plan


def plan_all(edge_index, cfg: Cfg):
    """Shard edges. Returns (offsets per core, p2 plans per core)."""
    row = np.asarray(edge_index[0]).astype(np.int64)
    col = np.asarray(edge_index[1]).astype(np.int64)

    # ---- L1: per-core owned-col offsets (host indexing only) ----
    owner = col // cfg.rpc_real
    offs = []
    for cr in range(cfg.ncores):
        loc = np.sort(col[owner == cr] - cr * cfg.rpc_real)
        off = np.searchsorted(loc, np.arange(cfg.rpc + 1)).astype(np.float32)
        offs.append(off)

    # ---- L2: row shards ----
    rowner = row // cfg.rpc_real
    plans = []
    for cr in range(cfg.ncores):
        sel = rowner == cr
        plans.append(plan_core_p2((row[sel] - cr * cfg.rpc_real), col[sel], cfg))
    return offs, plans


# ----------------------------------------------------------------------------
# bass program builders
# ----------------------------------------------------------------------------

def _patch_act_tables(arch):
    """Steer the act-table chooser to the combined Ln+Exp set.

    The insert_act_table_loads pass picks the first act_info.json set
    containing each required function, which ping-pongs between the
    Ln-only and Exp-only sets (a ~1.3us table DMA per switch). Ln and Exp
    coexist in natural_log_exp_and_others; pruning them from the other
    sets (in the cached dict, preserving set ids) makes the chooser pick
    the combined set once. The emitted loads stay semantically correct —
    the chosen table genuinely contains every function used under it.
    """
    from concourse.hw_specs import get_activation_tables
    t = get_activation_tables(arch)
    combined = "natural_log_exp_and_others"
    if combined not in t:
        return
    for name, s in t.items():
        if name != combined:
            s.discard(ACT.Ln)
            s.discard(ACT.Exp)


def _mk_nc(**kw):
    nc = bacc.Bacc("TRN2", target_bir_lowering=False, debug=False,
                   enable_partition_id=False, **kw)
    _patch_act_tables(nc.m.arch)
    return nc


def _iota_tile():
    i = np.arange(128, dtype=np.float32)
    return np.tile(i, (128, 1)).astype(ml_dtypes.bfloat16)


def _dyn_loop(nc, tc, cpool, ctx, dynamic):
    if not dynamic:
        return
    rep_in = nc.dram_tensor("rep", [1, 1], I32, kind="ExternalInput")
    rep_sb = cpool.tile([1, 1], I32)
    nc.sync.dma_start(rep_sb[:], rep_in.ap()[:])
    regs = []
    for e in mybir.ALL_ENGINES:
        regs.append(nc.alloc_register(e, f"repreg_{e.name}"))
    nc.regs_load(bass.RegisterHandles(tuple(regs)), rep_sb[0:1, 0:1])
    rep_val = bass.make_scalar_value(
        bass.RegisterHandles(tuple(regs)), min_val=0, max_val=1 << 20)
    ctx.enter_context(tc.For_i(0, rep_val, 1))


def build_min(cfg: Cfg, rep=0):
    """Uniform SPMD program: x-slice min."""
    dynamic = rep == -1
    nc = _mk_nc()
    x = nc.dram_tensor("x_own", [cfg.rpc, 128], F32, kind="ExternalInput")
    xmin = nc.dram_tensor("xmin", [1, 1], F32, kind="ExternalOutput")
    NW = cfg.nwin
    with tile.TileContext(nc) as tc, ExitStack() as ctx:
        pool = ctx.enter_context(tc.tile_pool(name="m", bufs=3))
        cpool = ctx.enter_context(tc.tile_pool(name="mc", bufs=1))
        runmin = cpool.tile([128, 1], F32)
        _dyn_loop(nc, tc, cpool, ctx, dynamic)
        SW = 4
        nt4 = (NW // SW) * SW * 128
        xt4 = x.ap()[0:nt4, :].rearrange("(t w p) f -> t p w f", p=128, w=SW)
        xt3 = x.ap().rearrange("(w p) f -> w p f", p=128)
        nc.vector.memset(runmin[:], 1e30)
        for t in range(NW // SW):
            xt = pool.tile([128, SW, 128], F32)
            nc.sync.dma_start(xt[:], xt4[t])
            red = pool.tile([128, 1], F32)
            nc.vector.tensor_reduce(red[:], xt[:], mybir.AxisListType.XY, ALU.min)
            nc.vector.tensor_tensor(runmin[:], runmin[:], red[:], ALU.min)
        for w in range((NW // SW) * SW, NW):
            xt1 = pool.tile([128, 128], F32, name="xt1", tag="xt1")
            nc.sync.dma_start(xt1[:], xt3[w])
            red1 = pool.tile([128, 1], F32, name="red1", tag="red1")
            nc.vector.tensor_reduce(red1[:], xt1[:], mybir.AxisListType.X, ALU.min)
            nc.vector.tensor_tensor(runmin[:], runmin[:], red1[:], ALU.min)
        negmin = cpool.tile([128, 1], F32, name="negmin", tag="negmin")
        nc.vector.tensor_scalar(negmin[:], runmin[:], -1.0, None, ALU.mult)
        allmax = cpool.tile([128, 1], F32, name="allmax", tag="allmax")
        nc.gpsimd.partition_all_reduce(allmax[:], negmin[:], 128,
                                       bass_isa.ReduceOp.max)
        minv = cpool.tile([1, 1], F32, name="minv", tag="minv")
        nc.vector.tensor_scalar(minv[:], allmax[0:1, :], -1.0, None, ALU.mult)
        nc.sync.dma_start(xmin.ap()[:], minv[:])
    nc.compile()
    return nc


def build_y(cfg: Cfg, rep=0):
    """Uniform SPMD program: deg -> dis -> y (bf16) for the owned slice."""
    dynamic = rep == -1
    nc = _mk_nc()
    x = nc.dram_tensor("x_own", [cfg.rpc, 128], F32, kind="ExternalInput")
    off = nc.dram_tensor("off_own", [cfg.rpc + 128], F32, kind="ExternalInput")
    mu = nc.dram_tensor("mu", [1, 1], F32, kind="ExternalInput")
    p_in = nc.dram_tensor("p", [1, 1], F32, kind="ExternalInput")
    y = nc.dram_tensor("y_own", [cfg.rpc, 128], BF16, kind="ExternalOutput")
    dis_out = nc.dram_tensor("dis_own", [cfg.rpc], F32, kind="ExternalOutput")

    NW = cfg.nwin
    with tile.TileContext(nc) as tc, ExitStack() as ctx:
        pool = ctx.enter_context(tc.tile_pool(name="y", bufs=3))
        cpool = ctx.enter_context(tc.tile_pool(name="yc", bufs=1))

        # scalars
        psb = cpool.tile([1, 1], F32)
        nc.sync.dma_start(psb[:], p_in.ap()[:])
        sig = cpool.tile([1, 1], F32)
        nc.scalar.activation(sig[:], psb[:], ACT.Sigmoid)
        ppb = cpool.tile([128, 1], F32)
        nc.gpsimd.partition_broadcast(ppb[:], sig[:])
        pp_vec = cpool.tile([128, 1], F32)
        nc.vector.tensor_scalar(pp_vec[:], ppb[:], 1.0, None, ALU.add)
        musb = cpool.tile([1, 1], F32)
        nc.sync.dma_start(musb[:], mu.ap()[:])
        mub = cpool.tile([128, 1], F32)
        nc.gpsimd.partition_broadcast(mub[:], musb[:])
        cvec = cpool.tile([128, 1], F32)   # 1e-6 - mu
        nc.vector.tensor_scalar(cvec[:], mub[:], -1.0, EPS_NUM, ALU.mult, ALU.add)

        # ---- deg = off[n+1] - off[n]; ldis = -0.5*ln(max(deg,1)) - 100*(deg==0)
        offA = cpool.tile([128, NW], F32)
        nc.sync.dma_start(offA[:], off.ap()[1:cfg.rpc + 1]
                          .rearrange("(w p) -> p w", p=128))
        offB = cpool.tile([128, NW], F32)
        nc.sync.dma_start(offB[:], off.ap()[0:cfg.rpc]
                          .rearrange("(w p) -> p w", p=128))
        deg = cpool.tile([128, NW], F32)
        nc.vector.tensor_tensor(deg[:], offA[:], offB[:], ALU.subtract)
        mask = cpool.tile([128, NW], F32)
        nc.vector.tensor_scalar(mask[:], deg[:], 0.5, None, ALU.is_ge)
        degc = cpool.tile([128, NW], F32)
        nc.vector.tensor_scalar(degc[:], deg[:], 1.0, None, ALU.max)
        ldeg = cpool.tile([128, NW], F32)
        nc.scalar.activation(ldeg[:], degc[:], ACT.Ln)
        ldis = cpool.tile([128, NW], F32)
        nc.vector.tensor_scalar(ldis[:], ldeg[:], -0.5, None, ALU.mult)
        pen = cpool.tile([128, NW], F32)
        nc.vector.tensor_scalar(pen[:], mask[:], 100.0, -100.0, ALU.mult, ALU.add)
        nc.vector.tensor_tensor(ldis[:], ldis[:], pen[:], ALU.add)
        dis = cpool.tile([128, NW], F32)
        nc.scalar.activation(dis[:], ldis[:], ACT.Exp)
        nc.sync.dma_start(dis_out.ap().rearrange("(w p) -> p w", p=128), dis[:])

        _dyn_loop(nc, tc, cpool, ctx, dynamic)
        # y = exp(pp*ln(x - mu + eps) + ldis), 4-window supertiles
        SW = 4
        nt4 = (NW // SW) * SW * 128
        xt4 = x.ap()[0:nt4, :].rearrange("(t w p) f -> t p w f", p=128, w=SW)
        yt4 = y.ap()[0:nt4, :].rearrange("(t w p) f -> t p w f", p=128, w=SW)
        for t in range(NW // SW):
            xt = pool.tile([128, SW, 128], F32)
            nc.sync.dma_start(xt[:], xt4[t])
            t1 = pool.tile([128, SW, 128], F32)
            nc.scalar.activation(t1[:], xt[:], ACT.Ln, bias=cvec[:, 0:1])
            yt = pool.tile([128, SW, 128], BF16)
            for w in range(SW):
                nc.scalar.activation(yt[:, w, :], t1[:, w, :], ACT.Exp,
                                     bias=ldis[:, t * SW + w:t * SW + w + 1],
                                     scale=pp_vec[:, 0:1])
            nc.sync.dma_start(yt4[t], yt[:])
        xt3 = x.ap().rearrange("(w p) f -> w p f", p=128)
        yt3 = y.ap().rearrange("(w p) f -> w p f", p=128)
        for w in range((NW // SW) * SW, NW):
            xt1 = pool.tile([128, 128], F32, name="xt1", tag="xt1")
            nc.sync.dma_start(xt1[:], xt3[w])
            t1b = pool.tile([128, 128], F32, name="t1b", tag="t1b")
            nc.scalar.activation(t1b[:], xt1[:], ACT.Ln, bias=cvec[:, 0:1])
            ytb = pool.tile([128, 128], BF16, name="ytb", tag="ytb")
            nc.scalar.activation(ytb[:], t1b[:], ACT.Exp,
                                 bias=ldis[:, w:w + 1], scale=pp_vec[:, 0:1])
            nc.sync.dma_start(yt3[w], ytb[:])
    nc.compile()
    return nc


def build_p2(cfg: Cfg, plan: CorePlan, rep=0):
    """Per-core program: gather + one-hot matmul segment-sum + transform."""
    dynamic = rep == -1
    nc = _mk_nc(num_swdge_queues=4)
    NW, G = cfg.nwin, cfg.group
    y = nc.dram_tensor("y_full", [cfg.N, 128], BF16, kind="ExternalInput")
    nbat = max(plan.nbatches, 1)
    gidx = nc.dram_tensor("gth_idx", list(plan.idx_wrapped.shape), I16,
                          kind="ExternalInput")
    rloc = nc.dram_tensor("row_local", [128, nbat], BF16, kind="ExternalInput")
    iota_d = nc.dram_tensor("iota", [128, 128], BF16, kind="ExternalInput")
    x = nc.dram_tensor("x_own", [cfg.rpc, 128], F32, kind="ExternalInput")
    dis = nc.dram_tensor("dis_own", [cfg.rpc], F32, kind="ExternalInput")
    mu = nc.dram_tensor("mu", [1, 1], F32, kind="ExternalInput")
    p_in = nc.dram_tensor("p", [1, 1], F32, kind="ExternalInput")
    eps_in = nc.dram_tensor("eps", [1, 1], F32, kind="ExternalInput")
    out = nc.dram_tensor("out_own", [cfg.rpc, 128], F32, kind="ExternalOutput")

    with tile.TileContext(nc) as tc, ExitStack() as ctx:
        cpool = ctx.enter_context(tc.tile_pool(name="c", bufs=1))
        stg = ctx.enter_context(tc.tile_pool(name="stg", bufs=cfg.stage_bufs))
        ppool = ctx.enter_context(tc.tile_pool(name="ph", bufs=6))
        psum = ctx.enter_context(tc.tile_pool(name="ps", bufs=4, space="PSUM"))
        opool = ctx.enter_context(tc.tile_pool(name="op", bufs=4))

        # ---- constants / scalars ----
        idx_sb = cpool.tile(list(plan.idx_wrapped.shape), I16)
        nc.sync.dma_start(idx_sb[:], gidx.ap()[:])
        rl_sb = cpool.tile([128, nbat], BF16)
        nc.sync.dma_start(rl_sb[:], rloc.ap()[:])
        io_sb = cpool.tile([128, 128], BF16)
        nc.sync.dma_start(io_sb[:], iota_d.ap()[:])
        dis_sb = cpool.tile([128, NW], F32)
        nc.sync.dma_start(dis_sb[:], dis.ap().rearrange("(w p) -> p w", p=128))

        psb = cpool.tile([1, 1], F32)
        nc.sync.dma_start(psb[:], p_in.ap()[:])
        sig = cpool.tile([1, 1], F32)
        nc.scalar.activation(sig[:], psb[:], ACT.Sigmoid)
        pp1 = cpool.tile([1, 1], F32)
        nc.vector.tensor_scalar(pp1[:], sig[:], 1.0, None, ALU.add)
        ipps = cpool.tile([1, 1], F32)
        nc.vector.reciprocal(ipps[:], pp1[:])
        ipp_vec = cpool.tile([128, 1], F32)
        nc.gpsimd.partition_broadcast(ipp_vec[:], ipps[:])

        esb = cpool.tile([1, 1], F32)
        nc.sync.dma_start(esb[:], eps_in.ap()[:])
        eb = cpool.tile([128, 1], F32)
        nc.gpsimd.partition_broadcast(eb[:], esb[:])
        oneps = cpool.tile([128, 1], F32)
        nc.vector.tensor_scalar(oneps[:], eb[:], 1.0, None, ALU.add)
        musb = cpool.tile([1, 1], F32)
        nc.sync.dma_start(musb[:], mu.ap()[:])
        mu_vec = cpool.tile([128, 1], F32)
        nc.gpsimd.partition_broadcast(mu_vec[:], musb[:])
        epsv = cpool.tile([128, 1], F32)
        nc.vector.memset(epsv[:], EPS_NUM)

        loop_cm = ExitStack()
        if dynamic:
            rep_in = nc.dram_tensor("rep", [1, 1], I32, kind="ExternalInput")
            rep_sb = cpool.tile([1, 1], I32)
            nc.sync.dma_start(rep_sb[:], rep_in.ap()[:])
            regs = []
            for e in mybir.ALL_ENGINES:
                regs.append(nc.alloc_register(e, f"repreg_{e.name}"))
            nc.regs_load(bass.RegisterHandles(tuple(regs)), rep_sb[0:1, 0:1])
            rep_val = bass.make_scalar_value(
                bass.RegisterHandles(tuple(regs)), min_val=0, max_val=1 << 20)
            loop_cm.enter_context(tc.For_i(0, rep_val, 1))

        xt3 = x.ap().rearrange("(w p) f -> w p f", p=128)
        # per-grp accumulators (separate tiles -> no false WAR deps between
        # one grp's transform reads and other grps' flush writes)
        agg_t = [cpool.tile([128, G * 128], F32, name=f"agg{g}",
                            tag=f"agg{g}") for g in range(cfg.ngrp)]
        obuf = cpool.tile([128, NW * 128], F32, name="obuf", tag="obuf")
        # zero sources for overwrite-style TT flushes (TT never grabs the
        # DVE/GpSimd shared SBUF port, unlike memset/tensor_scalar, which
        # would starve SWDGE descriptor generation mid-gather)
        zcol = cpool.tile([128, 1], F32)
        nc.vector.memset(zcol[:], 0.0)
        ztile = cpool.tile([128, 128], F32)
        nc.vector.memset(ztile[:], 0.0)

        def transform_grp(gg):
            """Output transform for grp gg: agg slice -> obuf slice.

            All HBM output traffic is deferred to one final DMA — interleaved
            64KB writes between the random gather reads thrash the HBM
            read/write turnaround when two cores share a stack.
            """
            w0 = gg * G
            wn = min(w0 + G, NW) - w0
            xt = opool.tile([128, G, 128], F32, name="xt", tag="xt")
            nc.sync.dma_start(xt[:, 0:wn, :],
                              x.ap()[w0 * 128:(w0 + wn) * 128, :]
                              .rearrange("(w p) f -> p w f", p=128))
            for w in range(w0, w0 + wn):
                t1 = opool.tile([128, 128], F32, name="t1", tag="t1")
                nc.scalar.activation(t1[:],
                                     agg_t[gg][:, (w % G) * 128:
                                               (w % G + 1) * 128],
                                     ACT.Ln, bias=epsv[:, 0:1],
                                     scale=dis_sb[:, w:w + 1])
                t2 = opool.tile([128, 128], F32, name="t2", tag="t2")
                nc.scalar.activation(t2[:], t1[:], ACT.Exp,
                                     scale=ipp_vec[:, 0:1])
                xw = opool.tile([128, 128], F32, name="xw", tag="xw")
                nc.scalar.activation(xw[:], xt[:, w - w0, :], ACT.Identity,
                                     bias=mu_vec[:, 0:1], scale=oneps[:, 0:1])
                nc.vector.tensor_tensor(obuf[:, w * 128:(w + 1) * 128],
                                        t2[:], xw[:], ALU.add)

        with loop_cm:
            for ww in plan.untouched_wins:
                nc.vector.tensor_tensor(
                    agg_t[ww // G][:, (ww % G) * 128:(ww % G + 1) * 128],
                    ztile[:],
                    zcol[:, 0:1].broadcast_to([128, 128]), ALU.add)
            yap = y.ap()
            live_psum = {}
            for ci, (bank, bt0, cn) in enumerate(plan.chunks):
                stage = stg.tile([128, cfg.chunk, 128], BF16)
                s0 = bt0 * 128
                nc.gpsimd.dma_gather(
                    stage[:, 0:cn, :],
                    yap[bank * cfg.bank_rows:
                        min((bank + 1) * cfg.bank_rows, cfg.N), :],
                    idx_sb[:, s0 // 16: (s0 + cn * 128) // 16],
                    cn * 128, cn * 128, 128, elem_step=128,
                    queue_num=ci % 4, single_packet=False,
                )
                for k in range(cn):
                    bi = bt0 + k
                    gg, bb, w = plan.batches[bi]
                    key = (bb, gg)
                    if key not in live_psum:
                        live_psum[key] = psum.tile([128, G * 128], F32,
                                                   name="grp", tag="grp")
                    pt = live_psum[key]
                    P = ppool.tile([128, 128], BF16)
                    nc.vector.tensor_tensor(
                        P[:], io_sb[:],
                        rl_sb[:, bi:bi + 1].broadcast_to([128, 128]),
                        ALU.is_equal)
                    first = plan.first_touch[(bb, w)] == bi
                    last = plan.last_touch[(bb, w)] == bi
                    sl = pt[:, (w % G) * 128:(w % G) * 128 + 128]
                    nc.tensor.matmul(sl, P[:], stage[:, k, :],
                                     start=first, stop=last)
                    # flushes + transforms scheduled after this batch
                    for (fb, fg, wins) in plan.flushes.get(bi, []):
                        pt2 = live_psum.pop((fb, fg))
                        runs = []   # [start, end, is_first]
                        for ww, isf in wins:
                            if runs and ww == runs[-1][1] and \
                                    isf == runs[-1][2]:
                                runs[-1][1] = ww + 1
                            else:
                                runs.append([ww, ww + 1, isf])
                        for a, bnd, isf in runs:
                            fd = (bnd - a) * 128
                            psl = pt2[:, (a % G) * 128:(a % G) * 128 + fd]
                            asl = agg_t[a // G][:, (a % G) * 128:
                                                (a % G) * 128 + fd]
                            if isf:
                                # first flush of these windows: agg = psum + 0
                                nc.vector.tensor_tensor(
                                    asl, psl,
                                    zcol[:, 0:1].broadcast_to([128, fd]),
                                    ALU.add)
                            else:
                                nc.vector.tensor_tensor(
                                    asl, asl, psl, ALU.add)
                    for gg2 in plan.transforms.get(bi, []):
                        transform_grp(gg2)
            # grps with no batches at all still need output
            for gg in range(cfg.ngrp):
                if gg not in plan.grps_with_batches:
                    transform_grp(gg)
            nc.sync.dma_start(
                out.ap().rearrange("(w p) f -> p w f", p=128),
                obuf[:].rearrange("p (w f) -> p w f", f=128))
    nc.compile()
    return nc


# ----------------------------------------------------------------------------
# PJRT runners
# ----------------------------------------------------------------------------

def _io_names(nc):
    in_names, out_names, out_avals = [], [], []
    import jax
    for alloc in nc.m.functions[0].allocations:
        if not isinstance(alloc, mybir.MemoryLocationSet):
            continue
        name = alloc.memorylocations[0].name
        if alloc.kind == "ExternalInput":
            if nc.partition_id_tensor is not None and \
                    name == nc.partition_id_tensor.name:
                continue
            in_names.append(name)
        elif alloc.kind == "ExternalOutput":
            out_names.append(name)
            out_avals.append(jax.core.ShapedArray(
                tuple(alloc.tensor_shape), mybir.dt.np(alloc.dtype)))
    return in_names, out_names, out_avals


def run_spmd(nc, in_maps):
    """Uniform program on len(in_maps) cores (the stock shard_map path)."""
    from concourse import bass2jax
    return bass2jax.run_bass_via_pjrt(nc, in_maps, n_cores=len(in_maps))


class SingleRunner:
    """One program pinned to one device; supports async dispatch."""

    def __init__(self, nc, device):
        import jax
        from concourse.bass2jax import _bass_exec_p, install_neuronx_cc_hook
        install_neuronx_cc_hook()
        assert nc.partition_id_tensor is None, \
            "per-core programs must not use partition id"
        self.nc, self.device = nc, device
        self.in_names, self.out_names, self.out_avals = _io_names(nc)
        all_in = tuple(self.in_names + self.out_names)
        out_avals = tuple(self.out_avals)
        out_names = tuple(self.out_names)

        def _body(*args):
            outs = _bass_exec_p.bind(
                *args, out_avals=out_avals, in_names=all_in,
                out_names=out_names, lowering_input_output_aliases=(),
                sim_require_finite=True, sim_require_nnan=True, nc=nc)
            return tuple(outs)

        n_params = len(self.in_names)
        donate = tuple(range(n_params, n_params + len(out_names)))
        self.fn = jax.jit(_body, donate_argnums=donate, keep_unused=True)
        self._dev_inputs = None

    def put_inputs(self, in_map):
        import jax
        self._dev_inputs = [jax.device_put(np.asarray(in_map[n]), self.device)
                            for n in self.in_names]
        jax.block_until_ready(self._dev_inputs)

    def dispatch(self):
        import jax
        import jax.numpy as jnp
        zeros = [jnp.zeros(a.shape, a.dtype, device=self.device)
                 for a in self.out_avals]
        return self.fn(*self._dev_inputs, *zeros)

    def collect(self, futs):
        return {n: np.asarray(f) for n, f in zip(self.out_names, futs)}


# ----------------------------------------------------------------------------
# numpy emulation of the planned L2 schedule (host-side logic check only)
# ----------------------------------------------------------------------------

def emulate_p2(cfg, plan, y_full, x_own, dis_own, mu, pp, eps):
    yf = np.asarray(y_full).astype(np.float32)
    agg = np.zeros((cfg.rpc, 128), np.float32)
    for bi, (gg, bb, w) in enumerate(plan.batches):
        s0 = bi * 128
        idx = np.zeros(128, np.int64)
        for i in range(128):
            idx[i] = plan.idx_wrapped[(s0 + i) % 16, (s0 + i) // 16]
        gl = bb * cfg.bank_rows + idx
        rl = plan.row_local[:, bi]
        Pm = (rl[:, None] == np.arange(128)[None, :]).astype(np.float32)
        agg[w * 128:(w + 1) * 128, :] += Pm.T @ yf[gl]
    o = np.exp((1.0 / pp) * np.log(dis_own[:, None] * agg + EPS_NUM))
    return o + (1 + eps) * x_own + mu


# ----------------------------------------------------------------------------
# public entry
# ----------------------------------------------------------------------------

_CACHE = {}


def _setup_jax():
    import jax
    cache = "/tmp/jax_neff_cache"
    os.makedirs(cache, exist_ok=True)
    try:
        jax.config.update("jax_compilation_cache_dir", cache)
        jax.config.update("jax_persistent_cache_min_entry_size_bytes", -1)
        jax.config.update("jax_persistent_cache_min_compile_time_secs", 0.0)
    except Exception:
        pass


def _pad_rows(a, rows, fill):
    if a.shape[0] == rows:
        return np.ascontiguousarray(a)
    out = np.full((rows,) + a.shape[1:], fill, a.dtype)
    out[: a.shape[0]] = a
    return out


def kernel(x, eps, p, edge_index):
    import jax
    _setup_jax()
    cfg = Cfg()
    x = np.asarray(x, np.float32)
    eps = np.asarray(eps, np.float32).reshape(1, 1)
    p = np.asarray(p, np.float32).reshape(1, 1)
    edge_index = np.asarray(edge_index)
    assert x.shape == (cfg.N, 128)

    offs, plans = plan_all(edge_index, cfg)
    x_sl = [
        _pad_rows(x[c * cfg.rpc_real:(c + 1) * cfg.rpc_real], cfg.rpc, 1e30)
        for c in range(cfg.ncores)
    ]
    # offsets padded so deg=0 beyond rpc_real
    off_sl = [_pad_rows(offs[c], cfg.rpc + 128, offs[c][-1])
              for c in range(cfg.ncores)]
    iota = _iota_tile()
    devices = jax.devices()[: cfg.ncores]

    # ---- L0: x min ----
    key0 = ("min", cfg.N)
    if key0 not in _CACHE:
        _CACHE[key0] = build_min(cfg)
    res0 = run_spmd(_CACHE[key0], [{"x_own": x_sl[c]}
                                   for c in range(cfg.ncores)])
    mu = np.array(min(float(r["xmin"][0, 0]) for r in res0),
                  np.float32).reshape(1, 1)

    # ---- L1: y ----
    key1 = ("y", cfg.N)
    if key1 not in _CACHE:
        _CACHE[key1] = build_y(cfg)
    res1 = run_spmd(_CACHE[key1], [
        {"x_own": x_sl[c], "off_own": off_sl[c], "mu": mu, "p": p}
        for c in range(cfg.ncores)
    ])
    y_full = np.concatenate(
        [res1[c]["y_own"][: cfg.rpc_real] for c in range(cfg.ncores)], axis=0)
    dis_sl = [res1[c]["dis_own"] for c in range(cfg.ncores)]

    # ---- L2 ----
    outs = [None] * cfg.ncores
    runners = []
    for c in range(cfg.ncores):
        key2 = ("p2", cfg.N, cfg.E, c,
                hash(plans[c].idx_wrapped.tobytes()),
                hash(plans[c].row_local.tobytes()))
        if key2 not in _CACHE:
            _CACHE[key2] = build_p2(cfg, plans[c])
        runners.append(SingleRunner(_CACHE[key2], devices[c]))
    for c in range(cfg.ncores):
        runners[c].put_inputs({
            "y_full": y_full, "gth_idx": plans[c].idx_wrapped,
            "row_local": plans[c].row_local, "iota": iota,
            "x_own": x_sl[c], "dis_own": dis_sl[c],
            "mu": mu, "p": p, "eps": eps,
        })
    futs = [runners[c].dispatch() for c in range(cfg.ncores)]
    jax.block_until_ready(futs)
    for c in range(cfg.ncores):
        outs[c] = runners[c].collect(futs[c])["out_own"][: cfg.rpc_real]
    return np.concatenate(outs, axis=0)
